# revision 1
# baseline (speedup 1.0000x reference)
"""Trainium2 Bass kernel for nn_CNNInteractLayer (CNN interaction layer).

Math: for each episode b, s-row i, q-row j:
  out[b,i,j] = maxpool_L(relu(conv_k(concat(s[b,i], q[b,j])))) for k in 2..5
Key factorization: conv(concat(s,q)) = conv_s(s) + conv_q(q) + bias, so we
compute per-row convolutions once (25+13 rows per core instead of 625 pairs)
and form pairwise sums with a 0/1 selection matmul on the PE. The max over
the L=31 window runs on the vector engine straight out of PSUM.

Sharding: 8 cores = 4 episodes x 2 halves of the q-row range.
"""

import os
import sys

import numpy as np

for _p in ("/opt/trn_rl_repo",):
    if os.path.isdir(_p) and _p not in sys.path:
        sys.path.insert(0, _p)

# the bass runner needs the axon jax backend; don't let a cpu-only pin hide it
if "axon" not in os.environ.get("JAX_PLATFORMS", "axon"):
    os.environ.pop("JAX_PLATFORMS", None)

from concourse import bacc, bass, mybir, tile  # noqa: E402
from concourse.bass_utils import run_bass_kernel_spmd  # noqa: E402

# Problem dims (hardcoded per spec)
B, N, K, Q, L, D = 4, 5, 5, 5, 31, 512
NROW = N * K            # 25 s-rows per episode
NQROW = N * Q           # 25 q-rows per episode
JN = 13                 # q-rows per core (padded; odd cores use 12)
ROWSTR = L + 4          # padded row stride (pad 2 each side)
POS_S = NROW * ROWSTR   # 875 real positions; computed out to 876 (even chunks)
POS_Q = JN * ROWSTR     # 455; computed out to 456
SLAB_S = 876            # conv output slab width per channel chunk
SLAB_Q = 456
PS_COLS = 880           # input halo: 2 left + enough right for pos 875 + delta 2
PQ_COLS = 460
NCH = 600               # device channels: [k5 | k4 | k3 | k2] x 150
# delta (tap shift) groups; prefix-size in device channel order
DELTAS = [(-2, 300), (-1, 600), (0, 600), (1, 450), (2, 150)]
# emission order per d-chunk: full-coverage groups first so the first matmul
# of each PSUM accumulation group writes the full partition range
DORDER = [1, 2, 0, 3, 4]
WOFF = [0, 300, 900, 1500, 1950]  # packed col offset of each delta group
WSIDE = 2100
CC0 = [0, 128, 256, 384, 512]     # channel chunk starts
CCW = [128, 128, 128, 128, 88]
XROWS = 39                        # 25 s + 13 q + 1 bias
NPAIR = NROW * JN                 # 325
MCH = [(0, 109), (109, 108), (217, 108)]
PAD_OF_K = {2: 1, 3: 1, 4: 2, 5: 2}
ORD_OF_K = {5: 0, 4: 1, 3: 2, 2: 3}
# fp32r matmul requires an even moving-dim size
POSCH_S = [(0, 488), (488, 388)]
POSCH_Q = [(0, 456)]
SUBW = 496                        # pairwise n-subchunk: 16 channel groups

# chunk-major packed-W layout: per channel chunk, [side s | side q], each a
# concatenation of the valid delta groups' column slices for that chunk
def _chunk_tables():
    chw = []          # per-side width of each chunk block
    coloff = {}       # (cc, side, di) -> column offset in packed W
    off = 0
    for cc in range(5):
        c0 = CC0[cc]
        widths = []
        for di, (_, sz) in enumerate(DELTAS):
            w = min(128, sz - c0) if sz > c0 else 0
            widths.append(w)
        side_w = sum(widths)
        for side in range(2):
            p = off + side * side_w
            for di, w in enumerate(widths):
                if w:
                    coloff[(cc, side, di)] = p
                    p += w
        chw.append(side_w)
        off += 2 * side_w
    return chw, coloff


CHW, WCOL = _chunk_tables()
CHOFF = [sum(2 * w for w in CHW[:i]) for i in range(6)]

_PROG = None


def _sub_plan(cc):
    """(offset, width) n-subchunks within an X chunk + psum bank grouping."""
    ccw = CCW[cc]
    total = ccw * 31
    subs = []
    off = 0
    while off < total:
        w = min(SUBW, total - off)
        subs.append((off, w))
        off += w
    # groups of <=3 subchunks sharing one psum tile; equal width within group
    groups = []
    i = 0
    while i < len(subs):
        g = [i]
        while (
            len(g) < 3
            and i + len(g) < len(subs)
            and subs[i + len(g)][1] == subs[i][1]
        ):
            g.append(i + len(g))
        groups.append(g)
        i += len(g)
    return subs, groups


def _build_program():
    nc = bacc.Bacc("TRN2", target_bir_lowering=False, debug=False, num_devices=8)
    f32 = mybir.dt.float32
    f32r = mybir.dt.float32r

    ps_d = nc.dram_tensor("ps", [D, PS_COLS], f32r, kind="ExternalInput")
    pq_d = nc.dram_tensor("pq", [D, PQ_COLS], f32r, kind="ExternalInput")
    w_d = nc.dram_tensor("w", [D, 2 * WSIDE], f32r, kind="ExternalInput")
    a_d = nc.dram_tensor("a", [XROWS, NPAIR], f32r, kind="ExternalInput")
    bias_d = nc.dram_tensor("bias", [1, 5 * 128 * 31], f32r, kind="ExternalInput")
    x_dram = nc.dram_tensor("xstage", [XROWS - 1, 5 * 128 * 31], f32r)
    out_d = nc.dram_tensor("out", [NPAIR, NCH], f32, kind="ExternalOutput")

    with tile.TileContext(nc) as tc:
        with (
            tc.tile_pool(name="persist", bufs=1) as big,
            tc.tile_pool(name="xpool", bufs=3) as xpool,
            tc.tile_pool(name="redpool", bufs=3) as redpool,
            tc.tile_pool(name="convps", bufs=2, space="PSUM") as convps,
            tc.tile_pool(name="pwps", bufs=2, space="PSUM") as pwps,
        ):
            w_sb = big.tile([128, 4 * 2 * WSIDE], f32r, tag="w")
            ps_sb = big.tile([128, 4 * PS_COLS], f32r, tag="ps")
            pq_sb = big.tile([128, 4 * PQ_COLS], f32r, tag="pq")
            cs_sb = big.tile([128, 5 * SLAB_S], f32r, tag="cs")
            cq_sb = big.tile([128, 5 * SLAB_Q], f32r, tag="cq")
            a_sb = big.tile([XROWS, NPAIR], f32r, tag="a")

            # keep the PE busy during the input-DMA prologue so the HAM
            # clock gate is warm (2.4 GHz) when the first conv matmul lands
            warm_sb = big.tile([128, 512], mybir.dt.bfloat16, tag="warm")
            warm_ps = convps.tile([128, 488], f32, tag="conv")
            nc.vector.memset(warm_sb[:], 0.0)
            for _wi in range(80):
                nc.tensor.matmul(
                    warm_ps[0:128, 0:256],
                    lhsT=warm_sb[:, 0:128],
                    rhs=warm_sb[:, 0:256],
                    start=True,
                    stop=True,
                )

            def wload(cc):
                wd = w_d[:].rearrange("(d p) c -> p d c", p=128)
                ws = w_sb[:].rearrange("p (d c) -> p d c", c=2 * WSIDE)
                nc.sync.dma_start(
                    ws[:, :, CHOFF[cc] : CHOFF[cc + 1]],
                    wd[:, :, CHOFF[cc] : CHOFF[cc + 1]],
                )

            wload(0)
            ps3 = ps_sb[:].rearrange("p (d c) -> p d c", c=PS_COLS)
            pd3 = ps_d[:].rearrange("(d p) c -> p d c", p=128)
            nc.sync.dma_start(ps3[:, :, 0:496], pd3[:, :, 0:496])
            nc.sync.dma_start(ps3[:, :, 496:PS_COLS], pd3[:, :, 496:PS_COLS])
            nc.sync.dma_start(
                pq_sb[:].rearrange("p (d c) -> p d c", c=PQ_COLS),
                pq_d[:].rearrange("(d p) c -> p d c", p=128),
            )
            nc.sync.dma_start(a_sb[:], a_d[:])
            wload(1)

            def conv(cc, side):
                """Conv for channel chunk cc of one side -> conv_sb slab."""
                c0, ccw = CC0[cc], CCW[cc]
                src, dst, poschunks, cols, slab = (
                    (ps_sb, cs_sb, POSCH_S, PS_COLS, SLAB_S)
                    if side == 0
                    else (pq_sb, cq_sb, POSCH_Q, PQ_COLS, SLAB_Q)
                )
                for pos0, pw in poschunks:
                    psum = convps.tile([128, 488], f32, tag="conv")
                    mms = []
                    for d in range(4):
                        # first and last matmul of the accumulation group must
                        # cover the full partition range (start/stop semantics
                        # are per-element), so full-size delta groups bracket
                        order = DORDER if d < 3 else [1, 0, 3, 4, 2]
                        for di in order:
                            delta, sz = DELTAS[di]
                            if sz <= c0:
                                continue
                            wcc = min(ccw, sz - c0)
                            mms.append((d, di, delta, wcc))
                    for idx, (d, di, delta, wcc) in enumerate(mms):
                        lcol = d * 2 * WSIDE + WCOL[(cc, side, di)]
                        rcol = d * cols + pos0 + delta + 2
                        nc.tensor.matmul(
                            psum[0:wcc, 0:pw],
                            lhsT=w_sb[:, lcol : lcol + wcc],
                            rhs=src[:, rcol : rcol + pw],
                            start=(idx == 0),
                            stop=(idx == len(mms) - 1),
                        )
                    nc.scalar.copy(
                        dst[0:ccw, cc * slab + pos0 : cc * slab + pos0 + pw],
                        psum[0:ccw, 0:pw],
                    )

            def xevict(cc):
                """conv_sb -> DRAM staging in X[row, slot*31 + l] layout.

                One DMA per side: DRAM write APs have no partition-dim
                ordering constraint, so (p, r, l) iteration can scatter to
                row-major X. Keeps total DMA count (and per-DMA HWDGE fixed
                cost) low.
                """
                xc0 = cc * 128 * 31
                ccw = CCW[cc]
                nc.sync.dma_start(
                    bass.AP(
                        x_dram[:].tensor,
                        xc0,
                        [[31, ccw], [5 * 128 * 31, NROW], [1, 31]],
                    ),
                    bass.AP(
                        cs_sb[:].tensor,
                        cs_sb[:].offset + cc * SLAB_S + 2,
                        [[cs_sb[:].ap[0][0], ccw], [ROWSTR, NROW], [1, 31]],
                    ),
                )
                nc.sync.dma_start(
                    bass.AP(
                        x_dram[:].tensor,
                        NROW * 5 * 128 * 31 + xc0,
                        [[31, ccw], [5 * 128 * 31, JN], [1, 31]],
                    ),
                    bass.AP(
                        cq_sb[:].tensor,
                        cq_sb[:].offset + cc * SLAB_Q + 2,
                        [[cq_sb[:].ap[0][0], ccw], [ROWSTR, JN], [1, 31]],
                    ),
                )

            def xload(cc):
                """DRAM staging -> X tile [39, 3968] (contiguous rows)."""
                xc0 = cc * 128 * 31
                w = CCW[cc] * 31
                xt = xpool.tile([XROWS, 128 * 31], f32r, tag="x")
                nc.sync.dma_start(
                    xt[0 : XROWS - 1, 0:w],
                    x_dram[:, xc0 : xc0 + w],
                )
                nc.sync.dma_start(
                    xt[XROWS - 1 : XROWS, 0:w],
                    bias_d[0:1, xc0 : xc0 + w],
                )
                return xt

            def pairwise(cc, xt, reds):
                subs, groups = _sub_plan(cc)
                for mi, (moff, msz) in enumerate(MCH):
                    for g in groups:
                        pw = pwps.tile([109, 3, 512], f32, tag="pw")
                        for j, si in enumerate(g):
                            soff, sw = subs[si]
                            nc.tensor.matmul(
                                pw[0:msz, j : j + 1, 0:sw],
                                lhsT=a_sb[:, moff : moff + msz],
                                rhs=xt[:, soff : soff + sw],
                                start=True,
                                stop=True,
                            )
                        ng = len(g)
                        gsw = subs[g[0]][1]
                        ncols = ng * (gsw // 31)
                        cb = CC0[cc] + subs[g[0]][0] // 31
                        nc.vector.tensor_reduce(
                            reds[mi][0:msz, cb : cb + ncols],
                            pw[0:msz, 0:ng, 0:gsw].rearrange(
                                "p g (c l) -> p g c l", l=31
                            ),
                            axis=mybir.AxisListType.X,
                            op=mybir.AluOpType.max,
                        )

            reds = [
                redpool.tile([109, NCH], f32, tag="red", name=f"red{i}")
                for i in range(3)
            ]

            # software-pipelined emission: conv leads xbuild by 1 chunk,
            # pairwise lags conv by 2 chunks (keeps PE fed while X DMAs land)
            xts = {}
            conv(0, 0)
            conv(0, 1)
            xevict(0)
            xts[0] = xload(0)
            for cc in range(1, 5):
                if cc + 1 <= 4:
                    wload(cc + 1)
                conv(cc, 0)
                conv(cc, 1)
                xevict(cc)
                xts[cc] = xload(cc)
                pairwise(cc - 1, xts.pop(cc - 1), reds)
            pairwise(4, xts.pop(4), reds)

            for mi, (moff, msz) in enumerate(MCH):
                nc.scalar.activation(
                    reds[mi][0:msz, :],
                    reds[mi][0:msz, :],
                    mybir.ActivationFunctionType.Relu,
                )
                nc.sync.dma_start(
                    out_d[moff : moff + msz, :], reds[mi][0:msz, 0:NCH]
                )

    nc.compile()
    return nc


def get_program():
    global _PROG
    if _PROG is None:
        _PROG = _build_program()
    return _PROG


def build_inputs(s, q, ws, bs):
    """Host-side shard prep. ws/bs: dicts k -> w(150, 1024, k) / b(150,).

    Returns (in_maps, core_meta). Core c handles episode c//2, q-row half c%2.
    """
    s = np.asarray(s, dtype=np.float32).reshape(B, NROW, L, D)
    q = np.asarray(q, dtype=np.float32).reshape(B, NQROW, L, D)

    # packed weights [D, 2*2100]: per side, delta groups at WOFF offsets,
    # device channel order [k5|k4|k3|k2]
    wall = np.zeros((D, 2 * WSIDE), dtype=np.float32)
    bias_dev = np.zeros(NCH, dtype=np.float32)
    for k in (2, 3, 4, 5):
        blk = ORD_OF_K[k] * 150
        bias_dev[blk : blk + 150] = bs[k]
        for di, (delta, sz) in enumerate(DELTAS):
            t = delta + PAD_OF_K[k]
            if not (0 <= t < k):
                continue
            assert blk + 150 <= sz
            wall[:, WOFF[di] + blk : WOFF[di] + blk + 150] = ws[k][:, :D, t].T
            wall[:, WSIDE + WOFF[di] + blk : WSIDE + WOFF[di] + blk + 150] = (
                ws[k][:, D:, t].T
            )
    perm = np.zeros(2 * WSIDE, dtype=np.int64)
    for side in range(2):
        for di, (_, sz) in enumerate(DELTAS):
            for cc in range(5):
                c0 = CC0[cc]
                if sz <= c0:
                    continue
                w = min(128, sz - c0)
                newc = WCOL[(cc, side, di)]
                oldc = side * WSIDE + WOFF[di] + c0
                perm[newc : newc + w] = np.arange(oldc, oldc + w)
    wall = wall[:, perm]

    bias_pad = np.zeros(5 * 128 * 31, dtype=np.float32)
    bias_pad[: NCH * 31] = np.repeat(bias_dev, 31)
    bias_rep = bias_pad[None, :]

    amat = np.zeros((XROWS, NPAIR), dtype=np.float32)
    for i in range(NROW):
        for t in range(JN):
            p = i * JN + t
            amat[i, p] = 1.0
            amat[NROW + t, p] = 1.0
    amat[XROWS - 1, :] = 1.0

    in_maps = []
    for core in range(8):
        b, jh = core // 2, core % 2
        jidx = [min(jh * JN + t, NQROW - 1) for t in range(JN)]
        psa = np.zeros((D, PS_COLS), dtype=np.float32)
        pqa = np.zeros((D, PQ_COLS), dtype=np.float32)
        for r in range(NROW):
            psa[:, r * ROWSTR + 4 : r * ROWSTR + 4 + L] = s[b, r].T
        for t, j in enumerate(jidx):
            pqa[:, t * ROWSTR + 4 : t * ROWSTR + 4 + L] = q[b, j].T
        in_maps.append(
            {"ps": psa, "pq": pqa, "w": wall, "a": amat, "bias": bias_rep}
        )
    return in_maps


# device channel -> original output channel maps
_S_IDX = np.array(
    [(3 - g) * 150 + u for g in range(4) for u in range(75)], dtype=np.int64
)
_Q_IDX = _S_IDX + 75


def assemble_outputs(core_outs):
    """core_outs: list of 8 arrays [NPAIR, NCH] -> (s_out, q_out)."""
    s_out = np.empty((B, NROW, NQROW, 300), dtype=np.float32)
    q_out = np.empty((B, NROW, NQROW, 300), dtype=np.float32)
    for core in range(8):
        b, jh = core // 2, core % 2
        nj = JN if jh == 0 else NQROW - JN
        arr = np.ascontiguousarray(core_outs[core]).reshape(NROW, JN, NCH)
        s_out[b, :, jh * JN : jh * JN + nj] = arr[:, :nj][:, :, _S_IDX]
        q_out[b, :, jh * JN : jh * JN + nj] = arr[:, :nj][:, :, _Q_IDX]
    return s_out.reshape(-1, 300), q_out.reshape(-1, 300)


def kernel(s, q, w2, b2, w3, b3, w4, b4, w5, b5, B=4, N=5, K=5, Q=5, L=31):
    ws = {2: np.asarray(w2, np.float32), 3: np.asarray(w3, np.float32),
          4: np.asarray(w4, np.float32), 5: np.asarray(w5, np.float32)}
    bs = {2: np.asarray(b2, np.float32), 3: np.asarray(b3, np.float32),
          4: np.asarray(b4, np.float32), 5: np.asarray(b5, np.float32)}
    in_maps = build_inputs(s, q, ws, bs)
    nc = get_program()
    res = run_bass_kernel_spmd(nc, in_maps, list(range(8))).results
    return assemble_outputs([res[c]["out"] for c in range(8)])



# revision 31
# speedup vs baseline: 1.3013x; 1.3013x over previous
"""Trainium2 Bass kernel for nn_CNNInteractLayer (CNN interaction layer).

Math: for each episode b, s-row i, q-row j:
  out[b,i,j] = maxpool_L(relu(conv_k(concat(s[b,i], q[b,j])))) for k in 2..5
Key factorization: conv(concat(s,q)) = conv_s(s) + conv_q(q) + bias, so we
compute per-row convolutions once (25 s + 25 q rows per core instead of 625
pairs) and form pairwise sums with a 0/1 selection matmul on the PE. The max
over the L=31 window runs on the vector engine straight out of PSUM, with a
GPSIMD tensor_tensor pre-max assist on part of the groups.

Sharding: 8 cores = 4 episodes x 2 halves of the 600 conv output channels
(half 0 -> the s_out channels of each conv, half 1 -> the q_out channels).
Each core computes all 625 row pairs of its episode for its 300 channels;
no inter-core communication. All matmul operands are bf16 (PSUM stays fp32).
"""

import os
import sys

import numpy as np

for _p in ("/opt/trn_rl_repo",):
    if os.path.isdir(_p) and _p not in sys.path:
        sys.path.insert(0, _p)

# the bass runner needs the axon jax backend; don't let a cpu-only pin hide it
if "axon" not in os.environ.get("JAX_PLATFORMS", "axon"):
    os.environ.pop("JAX_PLATFORMS", None)

import ml_dtypes  # noqa: E402

from concourse import bacc, bass, mybir, tile  # noqa: E402
from concourse.bass_utils import run_bass_kernel_spmd  # noqa: E402

BF16 = ml_dtypes.bfloat16

# Problem dims (hardcoded per spec)
B, N, K, Q, L, D = 4, 5, 5, 5, 31, 512
NROW = 25               # s-rows per episode
NQROW = 25              # q-rows per episode
ROWSTR = L + 4          # padded row stride (pad 2 each side)
SLAB = 876              # conv output slab width per channel chunk
P_COLS = 880            # input halo: 2 left + enough right for pos 875
NCH = 300               # device channels per core: [k5 | k4 | k3 | k2] x 75
NPAIR = NROW * NQROW    # 625
XROWS = NROW + NQROW + 1  # 25 s + 25 q + 1 bias
XBLK = 128 * 31         # x staging stride per channel chunk
# delta (tap shift) groups; prefix-size in device channel order
DELTAS = [(-2, 150), (-1, 300), (0, 300), (1, 225), (2, 75)]
CC0 = [0, 128, 256]     # channel chunk starts
CCW = [128, 128, 44]
POSCH = [(0, 488), (488, 388)]  # conv position chunks (PSUM bank = 488 f32)
SUBW = 496              # pairwise n-subchunk: 16 channel groups
MCH = [(125 * i, 125) for i in range(5)]  # pair-tile chunks
PAD_OF_K = {2: 1, 3: 1, 4: 2, 5: 2}
ORD_OF_K = {5: 0, 4: 1, 3: 2, 2: 3}

# per-group reduce routes (GPSIMD is unusable on TRN2 for this: no PSUM
# access, no TensorTensor, InstPool is DVE-only):
#  direct: DVE reduces the psum group in one go
#  B: Act copies psum->SBUF bf16 (relu folded in), DVE runs a 2x-mode max
#     tree (31->16->8->4) + tiny reduce; a quad (4 adjacent groups = one
#     128-ch chunk) shares one tree
def routes_for(cc, mi):
    """(directs, b_units) for one pair-tile of one channel chunk."""
    if cc == 0:
        return [], [(0, 1, 2, 3)]
    if cc == 1:
        return ([], [(0, 1, 2, 3)]) if mi <= 2 else ([0, 1, 2, 3], [])
    return [0, 1], []


# chunk-major packed-W layout: per channel chunk, [side s | side q], each a
# concatenation of the valid delta groups' column slices for that chunk
def _chunk_tables():
    chw = []          # per-side width of each chunk block
    coloff = {}       # (cc, side, di) -> column offset in packed W
    off = 0
    for cc in range(3):
        c0 = CC0[cc]
        widths = []
        for _, sz in DELTAS:
            widths.append(min(CCW[cc], sz - c0) if sz > c0 else 0)
        side_w = sum(widths)
        for side in range(2):
            p = off + side * side_w
            for di, w in enumerate(widths):
                if w:
                    coloff[(cc, side, di)] = p
                    p += w
        chw.append(side_w)
        off += 2 * side_w
    return chw, coloff


CHW, WCOL = _chunk_tables()
CHOFF = [sum(2 * w for w in CHW[:i]) for i in range(4)]
WSIDE = sum(CHW)        # 1050
WTOT = 2 * WSIDE        # 2100

_PROG = None


def _sub_plan(cc):
    """(offset, width) n-subchunks within an X chunk + psum tile grouping."""
    total = CCW[cc] * 31
    subs = []
    off = 0
    while off < total:
        w = min(SUBW, total - off)
        subs.append((off, w))
        off += w
    # groups of <=2 subchunks sharing one psum tile; equal width within group
    groups = []
    i = 0
    while i < len(subs):
        g = [i]
        while (
            len(g) < 2
            and i + len(g) < len(subs)
            and subs[i + len(g)][1] == subs[i][1]
        ):
            g.append(i + len(g))
        groups.append(g)
        i += len(g)
    return subs, groups


def _build_program():
    nc = bacc.Bacc("TRN2", target_bir_lowering=False, debug=False, num_devices=8)
    f32 = mybir.dt.float32
    f32r = mybir.dt.float32r
    bf16 = mybir.dt.bfloat16

    ps_d = nc.dram_tensor("ps", [D, P_COLS], bf16, kind="ExternalInput")
    pq_d = nc.dram_tensor("pq", [D, P_COLS], bf16, kind="ExternalInput")
    w_d = nc.dram_tensor("w", [D, WTOT], bf16, kind="ExternalInput")
    a_d = nc.dram_tensor("a", [XROWS, NPAIR], bf16, kind="ExternalInput")
    bias_d = nc.dram_tensor("bias", [1, 3 * XBLK], bf16, kind="ExternalInput")
    x_dram = nc.dram_tensor("xstage", [XROWS - 1, 3 * XBLK], bf16)
    out_d = nc.dram_tensor("out", [NPAIR, NCH], f32, kind="ExternalOutput")

    with tile.TileContext(nc) as tc:
        with (
            tc.tile_pool(name="persist", bufs=1) as big,
            tc.tile_pool(name="xpool", bufs=4) as xpool,
            tc.tile_pool(name="redpool", bufs=5) as redpool,
            tc.tile_pool(name="scrpool", bufs=4) as scrpool,
            tc.tile_pool(name="scr2pool", bufs=4) as scr2pool,
            tc.tile_pool(name="convps", bufs=2, space="PSUM") as convps,
            tc.tile_pool(name="pwps", bufs=3, space="PSUM") as pwps,
        ):
            w_sb = big.tile([128, 4 * WTOT], bf16, tag="w")
            ps_sb = big.tile([128, 4 * P_COLS], bf16, tag="ps")
            pq_sb = big.tile([128, 4 * P_COLS], bf16, tag="pq")
            cs_sb = big.tile([128, 3 * SLAB], bf16, tag="cs")
            cq_sb = big.tile([128, 3 * SLAB], bf16, tag="cq")
            a_sb = big.tile([XROWS, NPAIR], bf16, tag="a")

            # keep the PE busy during the input-DMA prologue so the HAM
            # clock gate is warm (2.4 GHz) when the first conv matmul lands
            warm_sb = big.tile([128, 512], bf16, tag="warm")
            warm_ps = convps.tile([128, 488], f32, tag="conv")
            nc.vector.memset(warm_sb[:, 0:256], 0.0)
            for _wi in range(26):
                nc.tensor.matmul(
                    warm_ps[0:128, 0:256],
                    lhsT=warm_sb[:, 0:128],
                    rhs=warm_sb[:, 0:256],
                    start=True,
                    stop=True,
                )

            def wload(cc):
                wd = w_d[:].rearrange("(d p) c -> p d c", p=128)
                ws = w_sb[:].rearrange("p (d c) -> p d c", c=WTOT)
                nc.sync.dma_start(
                    ws[:, :, CHOFF[cc] : CHOFF[cc + 1]],
                    wd[:, :, CHOFF[cc] : CHOFF[cc + 1]],
                )

            wload(1)
            ps3 = ps_sb[:].rearrange("p (d c) -> p d c", c=P_COLS)
            pd3 = ps_d[:].rearrange("(d p) c -> p d c", p=128)
            nc.sync.dma_start(ps3[:, :, 0:496], pd3[:, :, 0:496])
            nc.sync.dma_start(ps3[:, :, 496:P_COLS], pd3[:, :, 496:P_COLS])
            nc.sync.dma_start(
                pq_sb[:].rearrange("p (d c) -> p d c", c=P_COLS),
                pq_d[:].rearrange("(d p) c -> p d c", p=128),
            )
            nc.sync.dma_start(a_sb[:], a_d[:])
            wload(0)
            wload(2)

            def conv_unit(cc, side, pci):
                """Conv for channel chunk cc, one side, one position chunk.

                """
                pos0, pwid = POSCH[pci]
                src = ps_sb if side == 0 else pq_sb
                c0, ccw = CC0[cc], CCW[cc]
                mms = []
                for d in range(4):
                    for di, (delta, sz) in enumerate(DELTAS):
                        if sz <= c0:
                            continue
                        mms.append((d, di, delta, min(ccw, sz - c0)))
                # first and last matmul of the accumulation group must cover
                # the full partition range (start/stop are per-element)
                full = [m for m in mms if m[3] == ccw]
                part = [m for m in mms if m[3] < ccw]
                mms = [full[0]] + part + full[1:]
                psum = convps.tile([128, 488], f32, tag="conv")
                for idx, (d, di, delta, wcc) in enumerate(mms):
                    lcol = d * WTOT + WCOL[(cc, side, di)]
                    rcol = d * P_COLS + pos0 + delta + 2
                    nc.tensor.matmul(
                        psum[0:wcc, 0:pwid],
                        lhsT=w_sb[:, lcol : lcol + wcc],
                        rhs=src[:, rcol : rcol + pwid],
                        start=(idx == 0),
                        stop=(idx == len(mms) - 1),
                    )
                dst = cs_sb if side == 0 else cq_sb
                nc.scalar.copy(
                    dst[0:ccw, cc * SLAB + pos0 : cc * SLAB + pos0 + pwid],
                    psum[0:ccw, 0:pwid],
                )
                if pci == 1:
                    xevict_side(cc, side)

            def xevict_side(cc, side):
                """one side's conv slab -> DRAM staging X[row, slot*31+l]."""
                xc0 = cc * XBLK
                ccw = CCW[cc]
                src_sb = cs_sb if side == 0 else cq_sb
                nc.sync.dma_start(
                    bass.AP(
                        x_dram[:].tensor,
                        side * NROW * 3 * XBLK + xc0,
                        [[31, ccw], [3 * XBLK, NROW], [1, 31]],
                    ),
                    bass.AP(
                        src_sb[:].tensor,
                        src_sb[:].offset + cc * SLAB + 2,
                        [[src_sb[:].ap[0][0], ccw], [ROWSTR, NROW], [1, 31]],
                    ),
                )

            def xload(cc):
                """DRAM staging -> X tiles [51, <=1984] (left/right halves).

                Two tiles so the first pairwise groups of a chunk only wait
                on the left half's DMA.
                """
                xc0 = cc * XBLK
                w = CCW[cc] * 31
                halves = []
                for h0 in range(0, w, 1984):
                    hw = min(1984, w - h0)
                    xt = xpool.tile([XROWS, 1984], bf16, tag="x")
                    nc.sync.dma_start(
                        xt[0 : XROWS - 1, 0:hw],
                        x_dram[:, xc0 + h0 : xc0 + h0 + hw],
                    )
                    nc.sync.dma_start(
                        xt[XROWS - 1 : XROWS, 0:hw],
                        bias_d[0:1, xc0 + h0 : xc0 + h0 + hw],
                    )
                    halves.append(xt)
                return halves

            def pw_matmuls(cc, xt, mi, g):
                subs, _ = _sub_plan(cc)
                moff, msz = MCH[mi]
                pw = pwps.tile([125, 2, 512], f32, tag="pw")
                for j, si in enumerate(g):
                    soff, sw = subs[si]
                    xh = xt[soff // 1984]
                    nc.tensor.matmul(
                        pw[0:msz, j : j + 1, 0:sw],
                        lhsT=a_sb[:, moff : moff + msz],
                        rhs=xh[:, soff % 1984 : soff % 1984 + sw],
                        start=True,
                        stop=True,
                    )
                ng = len(g)
                gsw = subs[g[0]][1]
                return pw[0:msz, 0:ng, 0:gsw].rearrange(
                    "p g (c l) -> p g c l", l=31
                )

            def grp_info(cc, mi, gi):
                subs, groups = _sub_plan(cc)
                g = groups[gi]
                gsw = subs[g[0]][1]
                gc = gsw // 31
                cb = CC0[cc] + subs[g[0]][0] // 31
                return g, gc, len(g) * gc, cb

            def pairwise_direct(cc, xt, mi, gi, reds):
                g, gc, ncols, cb = grp_info(cc, mi, gi)
                moff, msz = MCH[mi]
                pwr = pw_matmuls(cc, xt, mi, g)
                nc.vector.tensor_reduce(
                    reds[mi][0:msz, cb : cb + ncols],
                    pwr,
                    axis=mybir.AxisListType.X,
                    op=mybir.AluOpType.max,
                )

            def pairwise_b(cc, xt, mi, gis, reds):
                """B route for up to 4 adjacent groups: Act drains each psum
                tile to SBUF bf16 (relu folded in -- idempotent under the
                later finish relu), then DVE max-trees 31->16->8->4 in 2x
                mode and a small reduce writes the result tile."""
                moff, msz = MCH[mi]
                infos = [grp_info(cc, mi, gi) for gi in gis]
                tg = sum(len(i[0]) for i in infos)
                gc = infos[0][1]
                cb = infos[0][3]
                ncols = sum(i[2] for i in infos)
                sc2 = scr2pool.tile([125, 8, 16, 48], mybir.dt.bfloat16,
                                    tag="scr2")
                goff = 0
                for g, _, _, _ in infos:
                    pwr = pw_matmuls(cc, xt, mi, g)
                    nc.scalar.activation(
                        sc2[0:msz, goff : goff + len(g), 0:gc, 0:31],
                        pwr,
                        mybir.ActivationFunctionType.Relu,
                    )
                    goff += len(g)

                def _stage2():
                    s = sc2[0:msz, 0:tg, 0:gc, :]
                    nc.vector.tensor_tensor(
                        s[:, :, :, 32:48], s[:, :, :, 0:16], s[:, :, :, 15:31],
                        op=mybir.AluOpType.max,
                    )
                    nc.vector.tensor_tensor(
                        s[:, :, :, 0:8], s[:, :, :, 32:40], s[:, :, :, 40:48],
                        op=mybir.AluOpType.max,
                    )
                    nc.vector.tensor_tensor(
                        s[:, :, :, 8:12], s[:, :, :, 0:4], s[:, :, :, 4:8],
                        op=mybir.AluOpType.max,
                    )
                    nc.vector.tensor_reduce(
                        reds[mi][0:msz, cb : cb + ncols],
                        s[:, :, :, 8:12],
                        axis=mybir.AxisListType.X,
                        op=mybir.AluOpType.max,
                    )
                    return None

                return _stage2

            reds = [
                redpool.tile([125, NCH], f32, tag=f"red{i}", name=f"red{i}")
                for i in range(5)
            ]

            def pw_units(cc):
                units = []
                for mi in range(5):
                    d, b = routes_for(cc, mi)
                    for gi in d:
                        units.append(("d", cc, mi, (gi,)))
                    for q in b:
                        units.append(("b", cc, mi, q))
                return units

            def finish_cc_mi(cc, mi):
                # relu + output DMA for one pair-tile, one channel chunk's
                # columns -- lets finished column ranges drain early
                moff, msz = MCH[mi]
                c0 = CC0[cc]
                c1 = min(c0 + CCW[cc], NCH)
                nc.scalar.activation(
                    reds[mi][0:msz, c0:c1],
                    reds[mi][0:msz, c0:c1],
                    mybir.ActivationFunctionType.Relu,
                )
                nc.sync.dma_start(
                    out_d[moff : moff + msz, c0:c1], reds[mi][0:msz, c0:c1]
                )

            xts = {}
            tails = []      # stage-2 closures (GPSIMD work)
            tails3 = []     # stage-3 closures (DVE finals)

            def emit(kind, cc, mi, gis):
                if kind == "d":
                    pairwise_direct(cc, xts[cc], mi, gis[0], reds)
                    return
                tails.append(pairwise_b(cc, xts[cc], mi, gis, reds))

            def flush_tails():
                for t in tails3:
                    t()
                tails3.clear()
                for t in tails:
                    n = t()
                    if n is not None:
                        tails3.append(n)
                tails.clear()

            # conv cc1 first; prologue DMAs land under the warmup
            for side in (0, 1):
                for pci in (0, 1):
                    conv_unit(1, side, pci)
            xts[1] = xload(1)
            # conv cc0 interleaved with pairwise cc1
            q1 = pw_units(1)
            sched0 = [3, 3, 3, 2]
            qi = 0
            for ui, (side, pci) in enumerate(
                ((0, 0), (0, 1), (1, 0), (1, 1))
            ):
                conv_unit(0, side, pci)
                flush_tails()
                for u in q1[qi : qi + sched0[ui]]:
                    emit(*u)
                qi += sched0[ui]
            for u in q1[qi:]:
                emit(*u)
            xts[0] = xload(0)
            flush_tails()
            flush_tails()
            for mi in range(5):
                finish_cc_mi(1, mi)
            # conv cc2 interleaved with pairwise cc0; hold some cc0 groups
            # back as PE filler over the xload(2) latency
            q0 = pw_units(0)
            sched2 = [1, 1, 1, 1]
            qi = 0
            for ui, (side, pci) in enumerate(
                ((0, 0), (0, 1), (1, 0), (1, 1))
            ):
                conv_unit(2, side, pci)
                flush_tails()
                for u in q0[qi : qi + sched2[ui]]:
                    emit(*u)
                qi += sched2[ui]
            xts[2] = xload(2)
            for u in q0[qi:]:
                emit(*u)
            flush_tails()
            flush_tails()
            for mi in range(5):
                finish_cc_mi(0, mi)
            # tail: pairwise cc2 (direct DVE reduces, shortest chains)
            for mi in range(5):
                d2, _ = routes_for(2, mi)
                if mi == 4:
                    d2 = list(reversed(d2))
                for gi in d2:
                    emit("d", 2, mi, (gi,))
                flush_tails()
                if mi > 1:
                    finish_cc_mi(2, mi - 2)
            flush_tails()
            flush_tails()
            finish_cc_mi(2, 3)
            finish_cc_mi(2, 4)

    nc.compile()
    return nc


def get_program():
    global _PROG
    if _PROG is None:
        _PROG = _build_program()
    return _PROG


def build_inputs(s, q, ws, bs):
    """Host-side shard prep. ws/bs: dicts k -> w(150, 1024, k) / b(150,).

    Returns in_maps. Core c handles episode c//2, channel half c%2
    (half 0 = out-channels 0:75 of each conv, half 1 = 75:150).
    """
    s = np.asarray(s, dtype=np.float32).reshape(B, NROW, L, D)
    q = np.asarray(q, dtype=np.float32).reshape(B, NQROW, L, D)

    walls, biasxs = [], []
    for half in range(2):
        wall = np.zeros((D, WTOT), dtype=np.float32)
        bias_dev = np.zeros(NCH, dtype=np.float32)
        for k in (2, 3, 4, 5):
            g = ORD_OF_K[k]
            bias_dev[g * 75 : (g + 1) * 75] = bs[k][75 * half : 75 * half + 75]
        for di, (delta, sz) in enumerate(DELTAS):
            for side in range(2):
                wd = np.zeros((D, NCH), dtype=np.float32)
                for k in (2, 3, 4, 5):
                    t = delta + PAD_OF_K[k]
                    if not (0 <= t < k):
                        continue
                    g = ORD_OF_K[k]
                    wd[:, g * 75 : (g + 1) * 75] = ws[k][
                        75 * half : 75 * half + 75, side * D : (side + 1) * D, t
                    ].T
                for cc in range(3):
                    c0 = CC0[cc]
                    if sz <= c0:
                        continue
                    w = min(CCW[cc], sz - c0)
                    col = WCOL[(cc, side, di)]
                    wall[:, col : col + w] = wd[:, c0 : c0 + w]
        biasx = np.zeros(3 * XBLK, dtype=np.float32)
        for ch in range(NCH):
            cc = 2 if ch >= 256 else ch // 128
            p = cc * XBLK + (ch - CC0[cc]) * 31
            biasx[p : p + 31] = bias_dev[ch]
        walls.append(wall.astype(BF16))
        biasxs.append(biasx[None, :].astype(BF16))

    amat = np.zeros((XROWS, NPAIR), dtype=np.float32)
    for i in range(NROW):
        for j in range(NQROW):
            p = i * NQROW + j
            amat[i, p] = 1.0
            amat[NROW + j, p] = 1.0
    amat[XROWS - 1, :] = 1.0
    amat = amat.astype(BF16)

    psas, pqas = [], []
    for b in range(B):
        psa = np.zeros((D, P_COLS), dtype=np.float32)
        pqa = np.zeros((D, P_COLS), dtype=np.float32)
        for r in range(NROW):
            psa[:, r * ROWSTR + 4 : r * ROWSTR + 4 + L] = s[b, r].T
        for r in range(NQROW):
            pqa[:, r * ROWSTR + 4 : r * ROWSTR + 4 + L] = q[b, r].T
        psas.append(psa.astype(BF16))
        pqas.append(pqa.astype(BF16))

    in_maps = []
    for core in range(8):
        b, half = core // 2, core % 2
        in_maps.append(
            {
                "ps": psas[b],
                "pq": pqas[b],
                "w": walls[half],
                "a": amat,
                "bias": biasxs[half],
            }
        )
    return in_maps


# device channel -> original output-column map (an involution)
_IDX = np.array(
    [(3 - g) * 75 + u for g in range(4) for u in range(75)], dtype=np.int64
)


def assemble_outputs(core_outs):
    """core_outs: list of 8 arrays [NPAIR, NCH] -> (s_out, q_out)."""
    s_out = np.empty((B, NPAIR, 300), dtype=np.float32)
    q_out = np.empty((B, NPAIR, 300), dtype=np.float32)
    for core in range(8):
        b, half = core // 2, core % 2
        arr = np.ascontiguousarray(core_outs[core])
        dst = s_out if half == 0 else q_out
        dst[b] = arr[:, _IDX]
    return s_out.reshape(-1, 300), q_out.reshape(-1, 300)


def kernel(s, q, w2, b2, w3, b3, w4, b4, w5, b5, B=4, N=5, K=5, Q=5, L=31):
    ws = {2: np.asarray(w2, np.float32), 3: np.asarray(w3, np.float32),
          4: np.asarray(w4, np.float32), 5: np.asarray(w5, np.float32)}
    bs = {2: np.asarray(b2, np.float32), 3: np.asarray(b3, np.float32),
          4: np.asarray(b4, np.float32), 5: np.asarray(b5, np.float32)}
    in_maps = build_inputs(s, q, ws, bs)
    nc = get_program()
    res = run_bass_kernel_spmd(nc, in_maps, list(range(8))).results
    return assemble_outputs([res[c]["out"] for c in range(8)])


# revision 35
# speedup vs baseline: 1.3733x; 1.0553x over previous
"""Trainium2 Bass kernel for nn_CNNInteractLayer (CNN interaction layer).

Math: for each episode b, s-row i, q-row j:
  out[b,i,j] = maxpool_L(relu(conv_k(concat(s[b,i], q[b,j])))) for k in 2..5
Key factorization: conv(concat(s,q)) = conv_s(s) + conv_q(q) + bias, so we
compute per-row convolutions once (25 s + 25 q rows per core instead of 625
pairs) and form pairwise sums with a 0/1 selection matmul on the PE. The max
over the L=31 window runs on the vector engine straight out of PSUM, with a
GPSIMD tensor_tensor pre-max assist on part of the groups.

Sharding: 8 cores = 4 episodes x 2 halves of the 600 conv output channels
(half 0 -> the s_out channels of each conv, half 1 -> the q_out channels).
Each core computes all 625 row pairs of its episode for its 300 channels;
no inter-core communication. All matmul operands are bf16 (PSUM stays fp32).
"""

import os
import sys

import numpy as np

for _p in ("/opt/trn_rl_repo",):
    if os.path.isdir(_p) and _p not in sys.path:
        sys.path.insert(0, _p)

# the bass runner needs the axon jax backend; don't let a cpu-only pin hide it
if "axon" not in os.environ.get("JAX_PLATFORMS", "axon"):
    os.environ.pop("JAX_PLATFORMS", None)

import ml_dtypes  # noqa: E402

from concourse import bacc, bass, mybir, tile  # noqa: E402
from concourse.bass_utils import run_bass_kernel_spmd  # noqa: E402

BF16 = ml_dtypes.bfloat16

# Problem dims (hardcoded per spec)
B, N, K, Q, L, D = 4, 5, 5, 5, 31, 512
NROW = 25               # s-rows per episode
NQROW = 25              # q-rows per episode
ROWSTR = L + 4          # padded row stride (pad 2 each side)
SLAB = 876              # conv output slab width per channel chunk
P_COLS = 880            # input halo: 2 left + enough right for pos 875
NCH = 300               # device channels per core: [k5 | k4 | k3 | k2] x 75
NPAIR = NROW * NQROW    # 625
XROWS = NROW + NQROW + 1  # 25 s + 25 q + 1 bias
XBLK = 128 * 31         # x staging stride per channel chunk
# delta (tap shift) groups; prefix-size in device channel order
DELTAS = [(-2, 150), (-1, 300), (0, 300), (1, 225), (2, 75)]
CC0 = [0, 128, 256]     # channel chunk starts
CCW = [128, 128, 44]
POSCH = [(0, 488), (488, 388)]  # conv position chunks (PSUM bank = 488 f32)
SUBW = 496              # pairwise n-subchunk: 16 channel groups
MCH = [(125 * i, 125) for i in range(5)]  # pair-tile chunks
PAD_OF_K = {2: 1, 3: 1, 4: 2, 5: 2}
ORD_OF_K = {5: 0, 4: 1, 3: 2, 2: 3}

# per-group reduce routes (GPSIMD is unusable on TRN2 for this: no PSUM
# access, no TensorTensor, InstPool is DVE-only):
#  direct: DVE reduces the psum group in one go
#  B: Act copies psum->SBUF bf16 (relu folded in), DVE runs a 2x-mode max
#     tree (31->16->8->4) + tiny reduce; a quad (4 adjacent groups = one
#     128-ch chunk) shares one tree
def routes_for(cc, mi):
    """(directs, b_units) for one pair-tile of one channel chunk."""
    if cc == 0:
        return ([0], [(1, 2, 3)]) if mi % 2 == 0 else ([0, 1], [(2, 3)])
    if cc == 1:
        return ([], [(0, 1, 2, 3)]) if mi <= 2 else ([0, 1], [(2, 3)])
    return [0, 1], []


# chunk-major packed-W layout: per channel chunk, [side s | side q], each a
# concatenation of the valid delta groups' column slices for that chunk
def _chunk_tables():
    chw = []          # per-side width of each chunk block
    coloff = {}       # (cc, side, di) -> column offset in packed W
    off = 0
    for cc in range(3):
        c0 = CC0[cc]
        widths = []
        for _, sz in DELTAS:
            widths.append(min(CCW[cc], sz - c0) if sz > c0 else 0)
        side_w = sum(widths)
        for side in range(2):
            p = off + side * side_w
            for di, w in enumerate(widths):
                if w:
                    coloff[(cc, side, di)] = p
                    p += w
        chw.append(side_w)
        off += 2 * side_w
    return chw, coloff


CHW, WCOL = _chunk_tables()
CHOFF = [sum(2 * w for w in CHW[:i]) for i in range(4)]
WSIDE = sum(CHW)        # 1050
WTOT = 2 * WSIDE        # 2100

_PROG = None


def _sub_plan(cc):
    """(offset, width) n-subchunks within an X chunk + psum tile grouping."""
    total = CCW[cc] * 31
    subs = []
    off = 0
    while off < total:
        w = min(SUBW, total - off)
        subs.append((off, w))
        off += w
    # groups of <=2 subchunks sharing one psum tile; equal width within group
    groups = []
    i = 0
    while i < len(subs):
        g = [i]
        while (
            len(g) < 2
            and i + len(g) < len(subs)
            and subs[i + len(g)][1] == subs[i][1]
        ):
            g.append(i + len(g))
        groups.append(g)
        i += len(g)
    return subs, groups


def _build_program():
    nc = bacc.Bacc("TRN2", target_bir_lowering=False, debug=False, num_devices=8)
    f32 = mybir.dt.float32
    f32r = mybir.dt.float32r
    bf16 = mybir.dt.bfloat16

    ps_d = nc.dram_tensor("ps", [D, P_COLS], bf16, kind="ExternalInput")
    pq_d = nc.dram_tensor("pq", [D, P_COLS], bf16, kind="ExternalInput")
    w_d = nc.dram_tensor("w", [D, WTOT], bf16, kind="ExternalInput")
    a_d = nc.dram_tensor("a", [XROWS, NPAIR], bf16, kind="ExternalInput")
    bias_d = nc.dram_tensor("bias", [1, 3 * XBLK], bf16, kind="ExternalInput")
    x_dram = nc.dram_tensor("xstage", [XROWS - 1, 3 * XBLK], bf16)
    out_d = nc.dram_tensor("out", [NPAIR, NCH], f32, kind="ExternalOutput")

    with tile.TileContext(nc) as tc:
        with (
            tc.tile_pool(name="persist", bufs=1) as big,
            tc.tile_pool(name="xpool", bufs=4) as xpool,
            tc.tile_pool(name="redpool", bufs=5) as redpool,
            tc.tile_pool(name="scrpool", bufs=4) as scrpool,
            tc.tile_pool(name="scr2pool", bufs=4) as scr2pool,
            tc.tile_pool(name="convps", bufs=2, space="PSUM") as convps,
            tc.tile_pool(name="pwps", bufs=3, space="PSUM") as pwps,
        ):
            w_sb = big.tile([128, 4 * WTOT], bf16, tag="w")
            ps_sb = big.tile([128, 4 * P_COLS], bf16, tag="ps")
            pq_sb = big.tile([128, 4 * P_COLS], bf16, tag="pq")
            cs_sb = big.tile([128, 3 * SLAB], bf16, tag="cs")
            cq_sb = big.tile([128, 3 * SLAB], bf16, tag="cq")
            a_sb = big.tile([XROWS, NPAIR], bf16, tag="a")

            # keep the PE busy during the input-DMA prologue so the HAM
            # clock gate is warm (2.4 GHz) when the first conv matmul lands
            warm_sb = big.tile([128, 512], bf16, tag="warm")
            warm_ps = convps.tile([128, 488], f32, tag="conv")
            nc.vector.memset(warm_sb[:, 0:256], 0.0)
            for _wi in range(26):
                nc.tensor.matmul(
                    warm_ps[0:128, 0:256],
                    lhsT=warm_sb[:, 0:128],
                    rhs=warm_sb[:, 0:256],
                    start=True,
                    stop=True,
                )

            def wload(cc):
                wd = w_d[:].rearrange("(d p) c -> p d c", p=128)
                ws = w_sb[:].rearrange("p (d c) -> p d c", c=WTOT)
                nc.sync.dma_start(
                    ws[:, :, CHOFF[cc] : CHOFF[cc + 1]],
                    wd[:, :, CHOFF[cc] : CHOFF[cc + 1]],
                )

            wload(1)
            ps3 = ps_sb[:].rearrange("p (d c) -> p d c", c=P_COLS)
            pd3 = ps_d[:].rearrange("(d p) c -> p d c", p=128)
            nc.sync.dma_start(ps3[:, :, 0:496], pd3[:, :, 0:496])
            nc.sync.dma_start(ps3[:, :, 496:P_COLS], pd3[:, :, 496:P_COLS])
            nc.sync.dma_start(
                pq_sb[:].rearrange("p (d c) -> p d c", c=P_COLS),
                pq_d[:].rearrange("(d p) c -> p d c", p=128),
            )
            nc.sync.dma_start(a_sb[:], a_d[:])
            wload(0)
            wload(2)

            def conv_unit(cc, side, pci):
                """Conv for channel chunk cc, one side, one position chunk.

                """
                pos0, pwid = POSCH[pci]
                src = ps_sb if side == 0 else pq_sb
                c0, ccw = CC0[cc], CCW[cc]
                mms = []
                for d in range(4):
                    for di, (delta, sz) in enumerate(DELTAS):
                        if sz <= c0:
                            continue
                        mms.append((d, di, delta, min(ccw, sz - c0)))
                # first and last matmul of the accumulation group must cover
                # the full partition range (start/stop are per-element)
                full = [m for m in mms if m[3] == ccw]
                part = [m for m in mms if m[3] < ccw]
                mms = [full[0]] + part + full[1:]
                psum = convps.tile([128, 488], f32, tag="conv")
                for idx, (d, di, delta, wcc) in enumerate(mms):
                    lcol = d * WTOT + WCOL[(cc, side, di)]
                    rcol = d * P_COLS + pos0 + delta + 2
                    nc.tensor.matmul(
                        psum[0:wcc, 0:pwid],
                        lhsT=w_sb[:, lcol : lcol + wcc],
                        rhs=src[:, rcol : rcol + pwid],
                        start=(idx == 0),
                        stop=(idx == len(mms) - 1),
                    )
                dst = cs_sb if side == 0 else cq_sb
                nc.scalar.copy(
                    dst[0:ccw, cc * SLAB + pos0 : cc * SLAB + pos0 + pwid],
                    psum[0:ccw, 0:pwid],
                )
                if pci == 1:
                    xevict_side(cc, side)

            def xevict_side(cc, side):
                """one side's conv slab -> DRAM staging X[row, slot*31+l]."""
                xc0 = cc * XBLK
                ccw = CCW[cc]
                src_sb = cs_sb if side == 0 else cq_sb
                nc.sync.dma_start(
                    bass.AP(
                        x_dram[:].tensor,
                        side * NROW * 3 * XBLK + xc0,
                        [[31, ccw], [3 * XBLK, NROW], [1, 31]],
                    ),
                    bass.AP(
                        src_sb[:].tensor,
                        src_sb[:].offset + cc * SLAB + 2,
                        [[src_sb[:].ap[0][0], ccw], [ROWSTR, NROW], [1, 31]],
                    ),
                )

            def xload(cc):
                """DRAM staging -> X tiles [51, <=1984] (left/right halves).

                Two tiles so the first pairwise groups of a chunk only wait
                on the left half's DMA.
                """
                xc0 = cc * XBLK
                w = CCW[cc] * 31
                halves = []
                for h0 in range(0, w, 1984):
                    hw = min(1984, w - h0)
                    xt = xpool.tile([XROWS, 1984], bf16, tag="x")
                    nc.sync.dma_start(
                        xt[0 : XROWS - 1, 0:hw],
                        x_dram[:, xc0 + h0 : xc0 + h0 + hw],
                    )
                    nc.sync.dma_start(
                        xt[XROWS - 1 : XROWS, 0:hw],
                        bias_d[0:1, xc0 + h0 : xc0 + h0 + hw],
                    )
                    halves.append(xt)
                return halves

            def pw_matmuls(cc, xt, mi, g):
                subs, _ = _sub_plan(cc)
                moff, msz = MCH[mi]
                pw = pwps.tile([125, 2, 512], f32, tag="pw")
                for j, si in enumerate(g):
                    soff, sw = subs[si]
                    xh = xt[soff // 1984]
                    nc.tensor.matmul(
                        pw[0:msz, j : j + 1, 0:sw],
                        lhsT=a_sb[:, moff : moff + msz],
                        rhs=xh[:, soff % 1984 : soff % 1984 + sw],
                        start=True,
                        stop=True,
                    )
                ng = len(g)
                gsw = subs[g[0]][1]
                return pw[0:msz, 0:ng, 0:gsw].rearrange(
                    "p g (c l) -> p g c l", l=31
                )

            def grp_info(cc, mi, gi):
                subs, groups = _sub_plan(cc)
                g = groups[gi]
                gsw = subs[g[0]][1]
                gc = gsw // 31
                cb = CC0[cc] + subs[g[0]][0] // 31
                return g, gc, len(g) * gc, cb

            def pairwise_direct(cc, xt, mi, gi, reds):
                g, gc, ncols, cb = grp_info(cc, mi, gi)
                moff, msz = MCH[mi]
                pwr = pw_matmuls(cc, xt, mi, g)
                nc.vector.tensor_reduce(
                    reds[mi][0:msz, cb : cb + ncols],
                    pwr,
                    axis=mybir.AxisListType.X,
                    op=mybir.AluOpType.max,
                )

            def pairwise_b(cc, xt, mi, gis, reds):
                """B route for up to 4 adjacent groups: Act drains each psum
                tile to SBUF bf16 (relu folded in -- idempotent under the
                later finish relu), then DVE max-trees 31->16->8->4 in 2x
                mode and a small reduce writes the result tile."""
                moff, msz = MCH[mi]
                infos = [grp_info(cc, mi, gi) for gi in gis]
                tg = sum(len(i[0]) for i in infos)
                gc = infos[0][1]
                cb = infos[0][3]
                ncols = sum(i[2] for i in infos)
                sc2 = scr2pool.tile([125, 8, 16, 48], mybir.dt.bfloat16,
                                    tag="scr2")
                goff = 0
                for g, _, _, _ in infos:
                    pwr = pw_matmuls(cc, xt, mi, g)
                    nc.scalar.activation(
                        sc2[0:msz, goff : goff + len(g), 0:gc, 0:31],
                        pwr,
                        mybir.ActivationFunctionType.Relu,
                    )
                    goff += len(g)

                def _stage2():
                    s = sc2[0:msz, 0:tg, 0:gc, :]
                    nc.vector.tensor_tensor(
                        s[:, :, :, 32:48], s[:, :, :, 0:16], s[:, :, :, 15:31],
                        op=mybir.AluOpType.max,
                    )
                    nc.vector.tensor_tensor(
                        s[:, :, :, 0:8], s[:, :, :, 32:40], s[:, :, :, 40:48],
                        op=mybir.AluOpType.max,
                    )
                    nc.vector.tensor_tensor(
                        s[:, :, :, 8:12], s[:, :, :, 0:4], s[:, :, :, 4:8],
                        op=mybir.AluOpType.max,
                    )
                    nc.vector.tensor_reduce(
                        reds[mi][0:msz, cb : cb + ncols],
                        s[:, :, :, 8:12],
                        axis=mybir.AxisListType.X,
                        op=mybir.AluOpType.max,
                    )
                    return None

                return _stage2

            reds = [
                redpool.tile([125, NCH], f32, tag=f"red{i}", name=f"red{i}")
                for i in range(5)
            ]

            def pw_units(cc):
                units = []
                for mi in range(5):
                    d, b = routes_for(cc, mi)
                    for gi in d:
                        units.append(("d", cc, mi, (gi,)))
                    for q in b:
                        units.append(("b", cc, mi, q))
                return units

            def finish_cc_mi(cc, mi):
                # relu + output DMA for one pair-tile, one channel chunk's
                # columns -- lets finished column ranges drain early
                moff, msz = MCH[mi]
                c0 = CC0[cc]
                c1 = min(c0 + CCW[cc], NCH)
                nc.scalar.activation(
                    reds[mi][0:msz, c0:c1],
                    reds[mi][0:msz, c0:c1],
                    mybir.ActivationFunctionType.Relu,
                )
                nc.sync.dma_start(
                    out_d[moff : moff + msz, c0:c1], reds[mi][0:msz, c0:c1]
                )

            xts = {}
            tails = []      # stage-2 closures (GPSIMD work)
            tails3 = []     # stage-3 closures (DVE finals)

            def emit(kind, cc, mi, gis):
                if kind == "d":
                    pairwise_direct(cc, xts[cc], mi, gis[0], reds)
                    return
                tails.append(pairwise_b(cc, xts[cc], mi, gis, reds))

            def flush_tails():
                for t in tails3:
                    t()
                tails3.clear()
                for t in tails:
                    n = t()
                    if n is not None:
                        tails3.append(n)
                tails.clear()

            # conv cc1 first; prologue DMAs land under the warmup
            for side in (0, 1):
                for pci in (0, 1):
                    conv_unit(1, side, pci)
            xts[1] = xload(1)
            # conv cc0 interleaved with pairwise cc1
            q1 = pw_units(1)
            sched0 = [3, 3, 3, 2]
            qi = 0
            for ui, (side, pci) in enumerate(
                ((0, 0), (0, 1), (1, 0), (1, 1))
            ):
                conv_unit(0, side, pci)
                flush_tails()
                for u in q1[qi : qi + sched0[ui]]:
                    emit(*u)
                qi += sched0[ui]
            for u in q1[qi:]:
                emit(*u)
            xts[0] = xload(0)
            flush_tails()
            flush_tails()
            for mi in range(5):
                finish_cc_mi(1, mi)
            # conv cc2 interleaved with pairwise cc0; hold some cc0 groups
            # back as PE filler over the xload(2) latency
            q0 = pw_units(0)
            sched2 = [1, 1, 1, 1]
            qi = 0
            for ui, (side, pci) in enumerate(
                ((0, 0), (0, 1), (1, 0), (1, 1))
            ):
                conv_unit(2, side, pci)
                flush_tails()
                for u in q0[qi : qi + sched2[ui]]:
                    emit(*u)
                qi += sched2[ui]
            xts[2] = xload(2)
            for u in q0[qi:]:
                emit(*u)
            flush_tails()
            flush_tails()
            for mi in range(5):
                finish_cc_mi(0, mi)
            # tail: pairwise cc2 (direct DVE reduces, shortest chains)
            for mi in range(5):
                d2, _ = routes_for(2, mi)
                if mi == 4:
                    d2 = list(reversed(d2))
                for gi in d2:
                    emit("d", 2, mi, (gi,))
                flush_tails()
                if mi > 1:
                    finish_cc_mi(2, mi - 2)
            flush_tails()
            flush_tails()
            finish_cc_mi(2, 3)
            finish_cc_mi(2, 4)

    nc.compile()
    return nc


def get_program():
    global _PROG
    if _PROG is None:
        _PROG = _build_program()
    return _PROG


def build_inputs(s, q, ws, bs):
    """Host-side shard prep. ws/bs: dicts k -> w(150, 1024, k) / b(150,).

    Returns in_maps. Core c handles episode c//2, channel half c%2
    (half 0 = out-channels 0:75 of each conv, half 1 = 75:150).
    """
    s = np.asarray(s, dtype=np.float32).reshape(B, NROW, L, D)
    q = np.asarray(q, dtype=np.float32).reshape(B, NQROW, L, D)

    walls, biasxs = [], []
    for half in range(2):
        wall = np.zeros((D, WTOT), dtype=np.float32)
        bias_dev = np.zeros(NCH, dtype=np.float32)
        for k in (2, 3, 4, 5):
            g = ORD_OF_K[k]
            bias_dev[g * 75 : (g + 1) * 75] = bs[k][75 * half : 75 * half + 75]
        for di, (delta, sz) in enumerate(DELTAS):
            for side in range(2):
                wd = np.zeros((D, NCH), dtype=np.float32)
                for k in (2, 3, 4, 5):
                    t = delta + PAD_OF_K[k]
                    if not (0 <= t < k):
                        continue
                    g = ORD_OF_K[k]
                    wd[:, g * 75 : (g + 1) * 75] = ws[k][
                        75 * half : 75 * half + 75, side * D : (side + 1) * D, t
                    ].T
                for cc in range(3):
                    c0 = CC0[cc]
                    if sz <= c0:
                        continue
                    w = min(CCW[cc], sz - c0)
                    col = WCOL[(cc, side, di)]
                    wall[:, col : col + w] = wd[:, c0 : c0 + w]
        biasx = np.zeros(3 * XBLK, dtype=np.float32)
        for ch in range(NCH):
            cc = 2 if ch >= 256 else ch // 128
            p = cc * XBLK + (ch - CC0[cc]) * 31
            biasx[p : p + 31] = bias_dev[ch]
        walls.append(wall.astype(BF16))
        biasxs.append(biasx[None, :].astype(BF16))

    amat = np.zeros((XROWS, NPAIR), dtype=np.float32)
    for i in range(NROW):
        for j in range(NQROW):
            p = i * NQROW + j
            amat[i, p] = 1.0
            amat[NROW + j, p] = 1.0
    amat[XROWS - 1, :] = 1.0
    amat = amat.astype(BF16)

    psas, pqas = [], []
    for b in range(B):
        psa = np.zeros((D, P_COLS), dtype=np.float32)
        pqa = np.zeros((D, P_COLS), dtype=np.float32)
        for r in range(NROW):
            psa[:, r * ROWSTR + 4 : r * ROWSTR + 4 + L] = s[b, r].T
        for r in range(NQROW):
            pqa[:, r * ROWSTR + 4 : r * ROWSTR + 4 + L] = q[b, r].T
        psas.append(psa.astype(BF16))
        pqas.append(pqa.astype(BF16))

    in_maps = []
    for core in range(8):
        b, half = core // 2, core % 2
        in_maps.append(
            {
                "ps": psas[b],
                "pq": pqas[b],
                "w": walls[half],
                "a": amat,
                "bias": biasxs[half],
            }
        )
    return in_maps


# device channel -> original output-column map (an involution)
_IDX = np.array(
    [(3 - g) * 75 + u for g in range(4) for u in range(75)], dtype=np.int64
)


def assemble_outputs(core_outs):
    """core_outs: list of 8 arrays [NPAIR, NCH] -> (s_out, q_out)."""
    s_out = np.empty((B, NPAIR, 300), dtype=np.float32)
    q_out = np.empty((B, NPAIR, 300), dtype=np.float32)
    for core in range(8):
        b, half = core // 2, core % 2
        arr = np.ascontiguousarray(core_outs[core])
        dst = s_out if half == 0 else q_out
        dst[b] = arr[:, _IDX]
    return s_out.reshape(-1, 300), q_out.reshape(-1, 300)


def kernel(s, q, w2, b2, w3, b3, w4, b4, w5, b5, B=4, N=5, K=5, Q=5, L=31):
    ws = {2: np.asarray(w2, np.float32), 3: np.asarray(w3, np.float32),
          4: np.asarray(w4, np.float32), 5: np.asarray(w5, np.float32)}
    bs = {2: np.asarray(b2, np.float32), 3: np.asarray(b3, np.float32),
          4: np.asarray(b4, np.float32), 5: np.asarray(b5, np.float32)}
    in_maps = build_inputs(s, q, ws, bs)
    nc = get_program()
    res = run_bass_kernel_spmd(nc, in_maps, list(range(8))).results
    return assemble_outputs([res[c]["out"] for c in range(8)])


# revision 47
# speedup vs baseline: 1.3856x; 1.0090x over previous
"""Trainium2 Bass kernel for nn_CNNInteractLayer (CNN interaction layer).

Math: for each episode b, s-row i, q-row j:
  out[b,i,j] = maxpool_L(relu(conv_k(concat(s[b,i], q[b,j])))) for k in 2..5
Key factorization: conv(concat(s,q)) = conv_s(s) + conv_q(q) + bias, so we
compute per-row convolutions once (25 s + 25 q rows per core instead of 625
pairs) and form pairwise sums with a 0/1 selection matmul on the PE. The max
over the L=31 window runs on the vector engine straight out of PSUM, with a
GPSIMD tensor_tensor pre-max assist on part of the groups.

Sharding: 8 cores = 4 episodes x 2 halves of the 600 conv output channels
(half 0 -> the s_out channels of each conv, half 1 -> the q_out channels).
Each core computes all 625 row pairs of its episode for its 300 channels;
no inter-core communication. All matmul operands are bf16 (PSUM stays fp32).
"""

import os
import sys

import numpy as np

for _p in ("/opt/trn_rl_repo",):
    if os.path.isdir(_p) and _p not in sys.path:
        sys.path.insert(0, _p)

# the bass runner needs the axon jax backend; don't let a cpu-only pin hide it
if "axon" not in os.environ.get("JAX_PLATFORMS", "axon"):
    os.environ.pop("JAX_PLATFORMS", None)

import ml_dtypes  # noqa: E402

from concourse import bacc, bass, mybir, tile  # noqa: E402
from concourse.bass_utils import run_bass_kernel_spmd  # noqa: E402

BF16 = ml_dtypes.bfloat16

# Problem dims (hardcoded per spec)
B, N, K, Q, L, D = 4, 5, 5, 5, 31, 512
NROW = 25               # s-rows per episode
NQROW = 25              # q-rows per episode
ROWSTR = L + 4          # padded row stride (pad 2 each side)
SLAB = 876              # conv output slab width per channel chunk
P_COLS = 880            # input halo: 2 left + enough right for pos 875
NCH = 300               # device channels per core: [k5 | k4 | k3 | k2] x 75
NPAIR = NROW * NQROW    # 625
XROWS = NROW + NQROW + 1  # 25 s + 25 q + 1 bias
XBLK = 128 * 31         # x staging stride per channel chunk
# delta (tap shift) groups; prefix-size in device channel order
DELTAS = [(-2, 150), (-1, 300), (0, 300), (1, 225), (2, 75)]
CC0 = [0, 128, 256]     # channel chunk starts
CCW = [128, 128, 44]
POSCH = [(0, 488), (488, 388)]  # conv position chunks (PSUM bank = 488 f32)
SUBW = 496              # pairwise n-subchunk: 16 channel groups
MCH = [(125 * i, 125) for i in range(5)]  # pair-tile chunks
PAD_OF_K = {2: 1, 3: 1, 4: 2, 5: 2}
ORD_OF_K = {5: 0, 4: 1, 3: 2, 2: 3}

# per-group reduce routes (GPSIMD is unusable on TRN2 for this: no PSUM
# access, no TensorTensor, InstPool is DVE-only):
#  direct: DVE reduces the psum group in one go
#  B: Act copies psum->SBUF bf16 (relu folded in), DVE runs a 2x-mode max
#     tree (31->16->8->4) + tiny reduce; a quad (4 adjacent groups = one
#     128-ch chunk) shares one tree
def routes_for(cc, mi):
    """(directs, b_units) for one pair-tile of one channel chunk."""
    if cc == 0:
        return ([0], [(1, 2, 3)]) if mi % 2 == 0 else ([0, 1], [(2, 3)])
    if cc == 1:
        return ([], [(0, 1, 2, 3)]) if mi <= 2 else ([0, 1], [(2, 3)])
    return [0, 1], []


# chunk-major packed-W layout: per channel chunk, [side s | side q], each a
# concatenation of the valid delta groups' column slices for that chunk
def _chunk_tables():
    chw = []          # per-side width of each chunk block
    coloff = {}       # (cc, side, di) -> column offset in packed W
    off = 0
    for cc in range(3):
        c0 = CC0[cc]
        widths = []
        for _, sz in DELTAS:
            widths.append(min(CCW[cc], sz - c0) if sz > c0 else 0)
        side_w = sum(widths)
        for side in range(2):
            p = off + side * side_w
            for di, w in enumerate(widths):
                if w:
                    coloff[(cc, side, di)] = p
                    p += w
        chw.append(side_w)
        off += 2 * side_w
    return chw, coloff


CHW, WCOL = _chunk_tables()
CHOFF = [sum(2 * w for w in CHW[:i]) for i in range(4)]
WSIDE = sum(CHW)        # 1050
WTOT = 2 * WSIDE        # 2100

_PROG = None


def _sub_plan(cc):
    """(offset, width) n-subchunks within an X chunk + psum tile grouping."""
    total = CCW[cc] * 31
    subs = []
    off = 0
    while off < total:
        w = min(SUBW, total - off)
        subs.append((off, w))
        off += w
    # groups of <=2 subchunks sharing one psum tile; equal width within group
    groups = []
    i = 0
    while i < len(subs):
        g = [i]
        while (
            len(g) < 2
            and i + len(g) < len(subs)
            and subs[i + len(g)][1] == subs[i][1]
        ):
            g.append(i + len(g))
        groups.append(g)
        i += len(g)
    return subs, groups


def _build_program():
    nc = bacc.Bacc("TRN2", target_bir_lowering=False, debug=False, num_devices=8)
    f32 = mybir.dt.float32
    f32r = mybir.dt.float32r
    bf16 = mybir.dt.bfloat16

    ps_d = nc.dram_tensor("ps", [D, P_COLS], bf16, kind="ExternalInput")
    pq_d = nc.dram_tensor("pq", [D, P_COLS], bf16, kind="ExternalInput")
    w_d = nc.dram_tensor("w", [D, WTOT], bf16, kind="ExternalInput")
    a_d = nc.dram_tensor("a", [XROWS, NPAIR], bf16, kind="ExternalInput")
    bias_d = nc.dram_tensor("bias", [1, 3 * XBLK], bf16, kind="ExternalInput")
    x_dram = nc.dram_tensor("xstage", [XROWS - 1, 3 * XBLK], bf16)
    out_d = nc.dram_tensor("out", [NPAIR, NCH], f32, kind="ExternalOutput")

    with tile.TileContext(nc) as tc:
        with (
            tc.tile_pool(name="persist", bufs=1) as big,
            tc.tile_pool(name="xpool", bufs=4) as xpool,
            tc.tile_pool(name="redpool", bufs=5) as redpool,
            tc.tile_pool(name="scrpool", bufs=4) as scrpool,
            tc.tile_pool(name="scr2pool", bufs=4) as scr2pool,
            tc.tile_pool(name="convps", bufs=2, space="PSUM") as convps,
            tc.tile_pool(name="pwps", bufs=3, space="PSUM") as pwps,
        ):
            w_sb = big.tile([128, 4 * WTOT], bf16, tag="w")
            ps_sb = big.tile([128, 4 * P_COLS], bf16, tag="ps")
            pq_sb = big.tile([128, 4 * P_COLS], bf16, tag="pq")
            cs_sb = big.tile([128, 3 * SLAB], bf16, tag="cs")
            cq_sb = big.tile([128, 3 * SLAB], bf16, tag="cq")
            a_sb = big.tile([XROWS, NPAIR], bf16, tag="a")

            # keep the PE busy during the input-DMA prologue so the HAM
            # clock gate is warm (2.4 GHz) when the first conv matmul lands
            warm_sb = big.tile([128, 512], bf16, tag="warm")
            warm_ps = convps.tile([128, 488], f32, tag="conv")
            nc.vector.memset(warm_sb[:, 0:256], 0.0)
            for _wi in range(20):
                nc.tensor.matmul(
                    warm_ps[0:128, 0:256],
                    lhsT=warm_sb[:, 0:128],
                    rhs=warm_sb[:, 0:256],
                    start=True,
                    stop=True,
                )

            def wload(cc):
                wd = w_d[:].rearrange("(d p) c -> p d c", p=128)
                ws = w_sb[:].rearrange("p (d c) -> p d c", c=WTOT)
                nc.sync.dma_start(
                    ws[:, :, CHOFF[cc] : CHOFF[cc + 1]],
                    wd[:, :, CHOFF[cc] : CHOFF[cc + 1]],
                )

            ps3 = ps_sb[:].rearrange("p (d c) -> p d c", c=P_COLS)
            pd3 = ps_d[:].rearrange("(d p) c -> p d c", p=128)
            nc.sync.dma_start(ps3[:, :, 0:496], pd3[:, :, 0:496])
            nc.sync.dma_start(ps3[:, :, 496:P_COLS], pd3[:, :, 496:P_COLS])
            # cc1's weights split per d-chunk so the first conv matmul (d0)
            # only waits on a quarter of the load
            wd_r = w_d[:].rearrange("(d p) c -> p d c", p=128)
            ws_r = w_sb[:].rearrange("p (d c) -> p d c", c=WTOT)
            for _d in range(4):
                nc.sync.dma_start(
                    ws_r[:, _d : _d + 1, CHOFF[1] : CHOFF[2]],
                    wd_r[:, _d : _d + 1, CHOFF[1] : CHOFF[2]],
                )
            nc.sync.dma_start(
                pq_sb[:].rearrange("p (d c) -> p d c", c=P_COLS),
                pq_d[:].rearrange("(d p) c -> p d c", p=128),
            )
            nc.sync.dma_start(a_sb[:], a_d[:])
            wload(0)
            wload(2)

            def conv_unit(cc, side, pci):
                """Conv for channel chunk cc, one side, one position chunk.

                """
                pos0, pwid = POSCH[pci]
                src = ps_sb if side == 0 else pq_sb
                c0, ccw = CC0[cc], CCW[cc]
                mms = []
                for d in range(4):
                    for di, (delta, sz) in enumerate(DELTAS):
                        if sz <= c0:
                            continue
                        mms.append((d, di, delta, min(ccw, sz - c0)))
                # first and last matmul of the accumulation group must cover
                # the full partition range (start/stop are per-element)
                full = [m for m in mms if m[3] == ccw]
                part = [m for m in mms if m[3] < ccw]
                mms = [full[0]] + part + full[1:]
                psum = convps.tile([128, 488], f32, tag="conv")
                for idx, (d, di, delta, wcc) in enumerate(mms):
                    lcol = d * WTOT + WCOL[(cc, side, di)]
                    rcol = d * P_COLS + pos0 + delta + 2
                    nc.tensor.matmul(
                        psum[0:wcc, 0:pwid],
                        lhsT=w_sb[:, lcol : lcol + wcc],
                        rhs=src[:, rcol : rcol + pwid],
                        start=(idx == 0),
                        stop=(idx == len(mms) - 1),
                    )
                dst = cs_sb if side == 0 else cq_sb
                nc.scalar.copy(
                    dst[0:ccw, cc * SLAB + pos0 : cc * SLAB + pos0 + pwid],
                    psum[0:ccw, 0:pwid],
                )
                if pci == 1:
                    xevict_side(cc, side)

            def xevict_side(cc, side):
                """one side's conv slab -> DRAM staging X[row, slot*31+l],
                split per channel half so each X half-tile's load chain
                starts as soon as its own half is staged."""
                xc0 = cc * XBLK
                ccw = CCW[cc]
                src_sb = cs_sb if side == 0 else cq_sb
                for h in range((ccw + 63) // 64):
                    c0 = h * 64
                    nch = min(64, ccw - c0)
                    nc.sync.dma_start(
                        bass.AP(
                            x_dram[:].tensor,
                            side * NROW * 3 * XBLK + xc0 + c0 * 31,
                            [[31, nch], [3 * XBLK, NROW], [1, 31]],
                        ),
                        bass.AP(
                            src_sb[:].tensor,
                            src_sb[:].offset + c0 * src_sb[:].ap[0][0]
                            + cc * SLAB + 2,
                            [[src_sb[:].ap[0][0], nch], [ROWSTR, NROW],
                             [1, 31]],
                        ),
                    )
                    if side == 1:
                        xload_half(cc, h)

            def xload_half(cc, h):
                """DRAM staging -> one X half tile (rows + bias row)."""
                xc0 = cc * XBLK
                w = CCW[cc] * 31
                h0 = h * 1984
                hw = min(1984, w - h0)
                xt = xts[cc][h]
                nc.sync.dma_start(
                    xt[0 : XROWS - 1, 0:hw],
                    x_dram[:, xc0 + h0 : xc0 + h0 + hw],
                )
                nc.sync.dma_start(
                    xt[XROWS - 1 : XROWS, 0:hw],
                    bias_d[0:1, xc0 + h0 : xc0 + h0 + hw],
                )

            def xalloc(cc):
                w = CCW[cc] * 31
                return [xpool.tile([XROWS, 1984], bf16, tag="x",
                                   name=f"xt{cc}_{h0}")
                        for h0 in range(0, w, 1984)]

            def pw_matmuls(cc, xt, mi, g):
                subs, _ = _sub_plan(cc)
                moff, msz = MCH[mi]
                pw = pwps.tile([125, 2, 512], f32, tag="pw")
                for j, si in enumerate(g):
                    soff, sw = subs[si]
                    xh = xt[soff // 1984]
                    nc.tensor.matmul(
                        pw[0:msz, j : j + 1, 0:sw],
                        lhsT=a_sb[:, moff : moff + msz],
                        rhs=xh[:, soff % 1984 : soff % 1984 + sw],
                        start=True,
                        stop=True,
                    )
                ng = len(g)
                gsw = subs[g[0]][1]
                return pw[0:msz, 0:ng, 0:gsw].rearrange(
                    "p g (c l) -> p g c l", l=31
                )

            def grp_info(cc, mi, gi):
                subs, groups = _sub_plan(cc)
                g = groups[gi]
                gsw = subs[g[0]][1]
                gc = gsw // 31
                cb = CC0[cc] + subs[g[0]][0] // 31
                return g, gc, len(g) * gc, cb

            def pairwise_direct(cc, xt, mi, gi, reds):
                g, gc, ncols, cb = grp_info(cc, mi, gi)
                moff, msz = MCH[mi]
                pwr = pw_matmuls(cc, xt, mi, g)
                nc.vector.tensor_reduce(
                    reds[mi][0:msz, cb : cb + ncols],
                    pwr,
                    axis=mybir.AxisListType.X,
                    op=mybir.AluOpType.max,
                )

            def pairwise_b(cc, xt, mi, gis, reds):
                """B route for up to 4 adjacent groups: Act drains each psum
                tile to SBUF bf16 (relu folded in -- idempotent under the
                later finish relu), then DVE max-trees 31->16->8->4 in 2x
                mode and a small reduce writes the result tile."""
                moff, msz = MCH[mi]
                infos = [grp_info(cc, mi, gi) for gi in gis]
                tg = sum(len(i[0]) for i in infos)
                gc = infos[0][1]
                cb = infos[0][3]
                ncols = sum(i[2] for i in infos)
                sc2 = scr2pool.tile([125, 8, 16, 48], mybir.dt.bfloat16,
                                    tag="scr2")
                goff = 0
                for g, _, _, _ in infos:
                    pwr = pw_matmuls(cc, xt, mi, g)
                    nc.scalar.activation(
                        sc2[0:msz, goff : goff + len(g), 0:gc, 0:31],
                        pwr,
                        mybir.ActivationFunctionType.Relu,
                    )
                    goff += len(g)

                def _stage2():
                    s = sc2[0:msz, 0:tg, 0:gc, :]
                    nc.vector.tensor_tensor(
                        s[:, :, :, 32:48], s[:, :, :, 0:16], s[:, :, :, 15:31],
                        op=mybir.AluOpType.max,
                    )
                    nc.vector.tensor_tensor(
                        s[:, :, :, 0:8], s[:, :, :, 32:40], s[:, :, :, 40:48],
                        op=mybir.AluOpType.max,
                    )
                    nc.vector.tensor_tensor(
                        s[:, :, :, 8:12], s[:, :, :, 0:4], s[:, :, :, 4:8],
                        op=mybir.AluOpType.max,
                    )
                    nc.vector.tensor_reduce(
                        reds[mi][0:msz, cb : cb + ncols],
                        s[:, :, :, 8:12],
                        axis=mybir.AxisListType.X,
                        op=mybir.AluOpType.max,
                    )
                    return None

                return _stage2

            reds = [
                redpool.tile([125, NCH], f32, tag=f"red{i}", name=f"red{i}")
                for i in range(5)
            ]

            def pw_units(cc):
                units = []
                for mi in range(5):
                    d, b = routes_for(cc, mi)
                    for gi in d:
                        units.append(("d", cc, mi, (gi,)))
                    for q in b:
                        units.append(("b", cc, mi, q))
                return units

            def finish_cc_mi(cc, mi):
                # relu + output DMA for one pair-tile, one channel chunk's
                # columns -- lets finished column ranges drain early
                moff, msz = MCH[mi]
                c0 = CC0[cc]
                c1 = min(c0 + CCW[cc], NCH)
                nc.scalar.activation(
                    reds[mi][0:msz, c0:c1],
                    reds[mi][0:msz, c0:c1],
                    mybir.ActivationFunctionType.Relu,
                )
                nc.sync.dma_start(
                    out_d[moff : moff + msz, c0:c1], reds[mi][0:msz, c0:c1]
                )

            xts = {}
            tails = []      # stage-2 closures (GPSIMD work)
            tails3 = []     # stage-3 closures (DVE finals)

            def emit(kind, cc, mi, gis):
                if kind == "d":
                    pairwise_direct(cc, xts[cc], mi, gis[0], reds)
                    return
                tails.append(pairwise_b(cc, xts[cc], mi, gis, reds))

            def flush_tails():
                for t in tails3:
                    t()
                tails3.clear()
                for t in tails:
                    n = t()
                    if n is not None:
                        tails3.append(n)
                tails.clear()

            # conv cc1 first; prologue DMAs land under the warmup
            xts[1] = xalloc(1)
            for side in (0, 1):
                for pci in (0, 1):
                    conv_unit(1, side, pci)
            # conv cc0 interleaved with pairwise cc1
            xts[0] = xalloc(0)
            q1 = pw_units(1)
            sched0 = [3, 3, 3, 2]
            qi = 0
            for ui, (side, pci) in enumerate(
                ((0, 0), (0, 1), (1, 0), (1, 1))
            ):
                conv_unit(0, side, pci)
                flush_tails()
                for u in q1[qi : qi + sched0[ui]]:
                    emit(*u)
                qi += sched0[ui]
            for u in q1[qi:]:
                emit(*u)
            flush_tails()
            flush_tails()
            for mi in range(5):
                finish_cc_mi(1, mi)
            # conv cc2 interleaved with pairwise cc0; hold some cc0 groups
            # back as PE filler over the xload(2) latency
            xts[2] = xalloc(2)
            q0 = pw_units(0)
            sched2 = [1, 1, 1, 1]
            qi = 0
            for ui, (side, pci) in enumerate(
                ((0, 0), (0, 1), (1, 0), (1, 1))
            ):
                conv_unit(2, side, pci)
                flush_tails()
                for u in q0[qi : qi + sched2[ui]]:
                    emit(*u)
                qi += sched2[ui]
            for u in q0[qi:]:
                emit(*u)
            flush_tails()
            flush_tails()
            for mi in range(5):
                finish_cc_mi(0, mi)
            # tail: pairwise cc2 (direct DVE reduces, shortest chains)
            for mi in range(5):
                d2, _ = routes_for(2, mi)
                if mi == 4:
                    d2 = list(reversed(d2))
                for gi in d2:
                    emit("d", 2, mi, (gi,))
                flush_tails()
                if mi > 1:
                    finish_cc_mi(2, mi - 2)
            flush_tails()
            flush_tails()
            finish_cc_mi(2, 3)
            finish_cc_mi(2, 4)

    nc.compile()
    return nc


def get_program():
    global _PROG
    if _PROG is None:
        _PROG = _build_program()
    return _PROG


def build_inputs(s, q, ws, bs):
    """Host-side shard prep. ws/bs: dicts k -> w(150, 1024, k) / b(150,).

    Returns in_maps. Core c handles episode c//2, channel half c%2
    (half 0 = out-channels 0:75 of each conv, half 1 = 75:150).
    """
    s = np.asarray(s, dtype=np.float32).reshape(B, NROW, L, D)
    q = np.asarray(q, dtype=np.float32).reshape(B, NQROW, L, D)

    walls, biasxs = [], []
    for half in range(2):
        wall = np.zeros((D, WTOT), dtype=np.float32)
        bias_dev = np.zeros(NCH, dtype=np.float32)
        for k in (2, 3, 4, 5):
            g = ORD_OF_K[k]
            bias_dev[g * 75 : (g + 1) * 75] = bs[k][75 * half : 75 * half + 75]
        for di, (delta, sz) in enumerate(DELTAS):
            for side in range(2):
                wd = np.zeros((D, NCH), dtype=np.float32)
                for k in (2, 3, 4, 5):
                    t = delta + PAD_OF_K[k]
                    if not (0 <= t < k):
                        continue
                    g = ORD_OF_K[k]
                    wd[:, g * 75 : (g + 1) * 75] = ws[k][
                        75 * half : 75 * half + 75, side * D : (side + 1) * D, t
                    ].T
                for cc in range(3):
                    c0 = CC0[cc]
                    if sz <= c0:
                        continue
                    w = min(CCW[cc], sz - c0)
                    col = WCOL[(cc, side, di)]
                    wall[:, col : col + w] = wd[:, c0 : c0 + w]
        biasx = np.zeros(3 * XBLK, dtype=np.float32)
        for ch in range(NCH):
            cc = 2 if ch >= 256 else ch // 128
            p = cc * XBLK + (ch - CC0[cc]) * 31
            biasx[p : p + 31] = bias_dev[ch]
        walls.append(wall.astype(BF16))
        biasxs.append(biasx[None, :].astype(BF16))

    amat = np.zeros((XROWS, NPAIR), dtype=np.float32)
    for i in range(NROW):
        for j in range(NQROW):
            p = i * NQROW + j
            amat[i, p] = 1.0
            amat[NROW + j, p] = 1.0
    amat[XROWS - 1, :] = 1.0
    amat = amat.astype(BF16)

    psas, pqas = [], []
    for b in range(B):
        psa = np.zeros((D, P_COLS), dtype=np.float32)
        pqa = np.zeros((D, P_COLS), dtype=np.float32)
        for r in range(NROW):
            psa[:, r * ROWSTR + 4 : r * ROWSTR + 4 + L] = s[b, r].T
        for r in range(NQROW):
            pqa[:, r * ROWSTR + 4 : r * ROWSTR + 4 + L] = q[b, r].T
        psas.append(psa.astype(BF16))
        pqas.append(pqa.astype(BF16))

    in_maps = []
    for core in range(8):
        b, half = core // 2, core % 2
        in_maps.append(
            {
                "ps": psas[b],
                "pq": pqas[b],
                "w": walls[half],
                "a": amat,
                "bias": biasxs[half],
            }
        )
    return in_maps


# device channel -> original output-column map (an involution)
_IDX = np.array(
    [(3 - g) * 75 + u for g in range(4) for u in range(75)], dtype=np.int64
)


def assemble_outputs(core_outs):
    """core_outs: list of 8 arrays [NPAIR, NCH] -> (s_out, q_out)."""
    s_out = np.empty((B, NPAIR, 300), dtype=np.float32)
    q_out = np.empty((B, NPAIR, 300), dtype=np.float32)
    for core in range(8):
        b, half = core // 2, core % 2
        arr = np.ascontiguousarray(core_outs[core])
        dst = s_out if half == 0 else q_out
        dst[b] = arr[:, _IDX]
    return s_out.reshape(-1, 300), q_out.reshape(-1, 300)


def kernel(s, q, w2, b2, w3, b3, w4, b4, w5, b5, B=4, N=5, K=5, Q=5, L=31):
    ws = {2: np.asarray(w2, np.float32), 3: np.asarray(w3, np.float32),
          4: np.asarray(w4, np.float32), 5: np.asarray(w5, np.float32)}
    bs = {2: np.asarray(b2, np.float32), 3: np.asarray(b3, np.float32),
          4: np.asarray(b4, np.float32), 5: np.asarray(b5, np.float32)}
    in_maps = build_inputs(s, q, ws, bs)
    nc = get_program()
    res = run_bass_kernel_spmd(nc, in_maps, list(range(8))).results
    return assemble_outputs([res[c]["out"] for c in range(8)])


# revision 60
# speedup vs baseline: 1.4350x; 1.0356x over previous
"""Trainium2 Bass kernel for nn_CNNInteractLayer (CNN interaction layer).

Math: for each episode b, s-row i, q-row j:
  out[b,i,j] = maxpool_L(relu(conv_k(concat(s[b,i], q[b,j])))) for k in 2..5
Key factorization: conv(concat(s,q)) = conv_s(s) + conv_q(q) + bias, so we
compute per-row convolutions once (25 s + 25 q rows per core instead of 625
pairs) and form pairwise sums with a 0/1 selection matmul on the PE. The max
over the L=31 window is split between DVE (direct tensor_reduce from PSUM)
and Act+DVE (relu-folded psum->SBUF bf16 copy, then a 2x-mode max tree),
keeping both PSUM-capable engines saturated.

Sharding: 8 cores = 4 episodes x 2 halves of the 600 conv output channels
(half 0 -> the s_out channels of each conv, half 1 -> the q_out channels).
Each core computes all 625 row pairs of its episode for its 300 channels;
no inter-core communication. All matmul operands are bf16 (PSUM stays fp32).
"""

import os
import sys

import numpy as np

for _p in ("/opt/trn_rl_repo",):
    if os.path.isdir(_p) and _p not in sys.path:
        sys.path.insert(0, _p)

# the bass runner needs the axon jax backend; don't let a cpu-only pin hide it
if "axon" not in os.environ.get("JAX_PLATFORMS", "axon"):
    os.environ.pop("JAX_PLATFORMS", None)

import ml_dtypes  # noqa: E402

from concourse import bacc, bass, mybir, tile  # noqa: E402
from concourse.bass_utils import run_bass_kernel_spmd  # noqa: E402

BF16 = ml_dtypes.bfloat16

# Problem dims (hardcoded per spec)
B, N, K, Q, L, D = 4, 5, 5, 5, 31, 512
NROW = 25               # s-rows per episode
NQROW = 25              # q-rows per episode
ROWSTR = L + 4          # padded row stride (pad 2 each side)
SLAB = 876              # halo'd conv output span per position chunk
SLABP = 775             # packed slab width per channel chunk (25*31)
P_COLS = 880            # input halo: 2 left + enough right for pos 875
NCH = 300               # device channels per core: [k5 | k4 | k3 | k2] x 75
NPAIR = NROW * NQROW    # 625
XROWS = NROW + NQROW + 1  # 25 s + 25 q + 1 bias
XBLK = 128 * 31         # x staging stride per channel chunk
# delta (tap shift) groups; prefix-size in device channel order
DELTAS = [(-2, 150), (-1, 300), (0, 300), (1, 225), (2, 75)]
CC0 = [0, 128, 256]     # channel chunk starts
CCW = [128, 128, 44]
POSCH = [(0, 488), (488, 388)]  # conv position chunks (PSUM bank = 488 f32)
SUBW = 496              # pairwise n-subchunk: 16 channel groups
MCH = [(125 * i, 125) for i in range(5)]  # pair-tile chunks
PAD_OF_K = {2: 1, 3: 1, 4: 2, 5: 2}
ORD_OF_K = {5: 0, 4: 1, 3: 2, 2: 3}

# per-group reduce routes (GPSIMD is unusable on TRN2 for this: no PSUM
# access, no TensorTensor, InstPool is DVE-only):
#  direct: DVE reduces the psum group in one go
#  B: Act copies psum->SBUF bf16 (relu folded in), DVE runs a 2x-mode max
#     tree (31->16->8->4) + tiny reduce; a quad (4 adjacent groups = one
#     128-ch chunk) shares one tree
def routes_for(cc, mi):
    """(directs, b_units) for one pair-tile of one channel chunk."""
    if cc == 0:
        return [0], [(1, 2, 3)]
    if cc == 1:
        return ([], [(0, 1, 2, 3)]) if mi <= 2 else ([0, 1], [(2, 3)])
    return [1], [(0,)]


# chunk-major packed-W layout: per channel chunk, [side s | side q], each a
# concatenation of the valid delta groups' column slices for that chunk
def _chunk_tables():
    chw = []          # per-side width of each chunk block
    coloff = {}       # (cc, side, di) -> column offset in packed W
    off = 0
    for cc in range(3):
        c0 = CC0[cc]
        widths = []
        for _, sz in DELTAS:
            widths.append(min(CCW[cc], sz - c0) if sz > c0 else 0)
        side_w = sum(widths)
        for side in range(2):
            p = off + side * side_w
            for di, w in enumerate(widths):
                if w:
                    coloff[(cc, side, di)] = p
                    p += w
        chw.append(side_w)
        off += 2 * side_w
    return chw, coloff


CHW, WCOL = _chunk_tables()
CHOFF = [sum(2 * w for w in CHW[:i]) for i in range(4)]
WSIDE = sum(CHW)        # 1050
WTOT = 2 * WSIDE        # 2100

_PROG = None


def _sub_plan(cc):
    """(offset, width) n-subchunks within an X chunk + psum tile grouping."""
    total = CCW[cc] * 31
    subs = []
    off = 0
    while off < total:
        w = min(SUBW, total - off)
        subs.append((off, w))
        off += w
    # groups of <=2 subchunks sharing one psum tile; equal width within group
    groups = []
    i = 0
    while i < len(subs):
        g = [i]
        while (
            len(g) < 2
            and i + len(g) < len(subs)
            and subs[i + len(g)][1] == subs[i][1]
        ):
            g.append(i + len(g))
        groups.append(g)
        i += len(g)
    return subs, groups


def _build_program():
    nc = bacc.Bacc("TRN2", target_bir_lowering=False, debug=False, num_devices=8)
    f32 = mybir.dt.float32
    f32r = mybir.dt.float32r
    bf16 = mybir.dt.bfloat16

    ps_d = nc.dram_tensor("ps", [D, P_COLS], bf16, kind="ExternalInput")
    pq_d = nc.dram_tensor("pq", [D, P_COLS], bf16, kind="ExternalInput")
    w_d = nc.dram_tensor("w", [D, WTOT], bf16, kind="ExternalInput")
    a_d = nc.dram_tensor("a", [XROWS, NPAIR], bf16, kind="ExternalInput")
    bias_d = nc.dram_tensor("bias", [1, 3 * XBLK], bf16, kind="ExternalInput")
    x_dram = nc.dram_tensor("xstage", [XROWS - 1, 3 * XBLK], bf16)
    out_d = nc.dram_tensor("out", [NPAIR, NCH], f32, kind="ExternalOutput")

    with tile.TileContext(nc) as tc:
        with (
            tc.tile_pool(name="persist", bufs=1) as big,
            tc.tile_pool(name="xpool", bufs=4) as xpool,
            tc.tile_pool(name="redpool", bufs=5) as redpool,
            tc.tile_pool(name="scrpool", bufs=4) as scrpool,
            tc.tile_pool(name="scr2pool", bufs=4) as scr2pool,
            tc.tile_pool(name="convps", bufs=2, space="PSUM") as convps,
            tc.tile_pool(name="pwps", bufs=3, space="PSUM") as pwps,
        ):
            w_sb = big.tile([128, 4 * WTOT], bf16, tag="w")
            ps_sb = big.tile([128, 4 * P_COLS], bf16, tag="ps")
            pq_sb = big.tile([128, 4 * P_COLS], bf16, tag="pq")
            cs_sb = big.tile([128, 3 * SLABP], bf16, tag="cs")
            cq_sb = big.tile([128, 3 * SLABP], bf16, tag="cq")
            a_sb = big.tile([XROWS, NPAIR], bf16, tag="a")

            # keep the PE busy during the input-DMA prologue so the HAM
            # clock gate is warm (2.4 GHz) when the first conv matmul lands
            warm_sb = big.tile([128, 512], bf16, tag="warm")
            warm_ps = convps.tile([128, 492], f32, tag="conv")
            nc.vector.memset(warm_sb[:, 0:256], 0.0)
            for _wi in range(20):
                nc.tensor.matmul(
                    warm_ps[0:128, 0:256],
                    lhsT=warm_sb[:, 0:128],
                    rhs=warm_sb[:, 0:256],
                    start=True,
                    stop=True,
                )

            def wload(cc):
                wd = w_d[:].rearrange("(d p) c -> p d c", p=128)
                ws = w_sb[:].rearrange("p (d c) -> p d c", c=WTOT)
                nc.sync.dma_start(
                    ws[:, :, CHOFF[cc] : CHOFF[cc + 1]],
                    wd[:, :, CHOFF[cc] : CHOFF[cc + 1]],
                )

            ps3 = ps_sb[:].rearrange("p (d c) -> p d c", c=P_COLS)
            pd3 = ps_d[:].rearrange("(d p) c -> p d c", p=128)
            nc.sync.dma_start(ps3[:, :, 0:496], pd3[:, :, 0:496])
            nc.sync.dma_start(ps3[:, :, 496:P_COLS], pd3[:, :, 496:P_COLS])
            # cc1's weights split per d-chunk so the first conv matmul (d0)
            # only waits on a quarter of the load
            wd_r = w_d[:].rearrange("(d p) c -> p d c", p=128)
            ws_r = w_sb[:].rearrange("p (d c) -> p d c", c=WTOT)
            for _d in range(4):
                nc.sync.dma_start(
                    ws_r[:, _d : _d + 1, CHOFF[1] : CHOFF[2]],
                    wd_r[:, _d : _d + 1, CHOFF[1] : CHOFF[2]],
                )
            nc.sync.dma_start(
                pq_sb[:].rearrange("p (d c) -> p d c", c=P_COLS),
                pq_d[:].rearrange("(d p) c -> p d c", p=128),
            )
            nc.sync.dma_start(a_sb[:], a_d[:])
            wload(0)
            wload(2)

            def conv_unit(cc, side, pci):
                """Conv for channel chunk cc, one side, one position chunk.

                """
                pos0, pwid = POSCH[pci]
                src = ps_sb if side == 0 else pq_sb
                c0, ccw = CC0[cc], CCW[cc]
                mms = []
                for d in range(4):
                    for di, (delta, sz) in enumerate(DELTAS):
                        if sz <= c0:
                            continue
                        mms.append((d, di, delta, min(ccw, sz - c0)))
                # first and last matmul of the accumulation group must cover
                # the full partition range (start/stop are per-element)
                full = [m for m in mms if m[3] == ccw]
                part = [m for m in mms if m[3] < ccw]
                mms = [full[0]] + part + full[1:]
                psum = convps.tile([128, 492], f32, tag="conv")
                for idx, (d, di, delta, wcc) in enumerate(mms):
                    lcol = d * WTOT + WCOL[(cc, side, di)]
                    rcol = d * P_COLS + pos0 + delta + 2
                    nc.tensor.matmul(
                        psum[0:wcc, 0:pwid],
                        lhsT=w_sb[:, lcol : lcol + wcc],
                        rhs=src[:, rcol : rcol + pwid],
                        start=(idx == 0),
                        stop=(idx == len(mms) - 1),
                    )
                dst = cs_sb if side == 0 else cq_sb
                # packed copy: strip the 4-col inter-row halo while draining
                r0, nr, lc = (0, 14, 2) if pci == 0 else (14, 11, 4)
                pr = psum[0:ccw, lc : lc + nr * ROWSTR].rearrange(
                    "p (r s) -> p r s", s=ROWSTR
                )
                nc.scalar.copy(
                    dst[0:ccw, cc * SLABP + r0 * 31 :
                        cc * SLABP + (r0 + nr) * 31],
                    pr[:, :, 0:31],
                )
                if pci == 1:
                    xevict_side(cc, side)

            def xevict_side(cc, side):
                """one side's conv slab -> DRAM staging X[row, slot*31+l],
                split per channel half so each X half-tile's load chain
                starts as soon as its own half is staged."""
                xc0 = cc * XBLK
                ccw = CCW[cc]
                src_sb = cs_sb if side == 0 else cq_sb
                for h in range((ccw + 63) // 64):
                    c0 = h * 64
                    nch = min(64, ccw - c0)
                    nc.sync.dma_start(
                        bass.AP(
                            x_dram[:].tensor,
                            side * NROW * 3 * XBLK + xc0 + c0 * 31,
                            [[31, nch], [3 * XBLK, NROW], [1, 31]],
                        ),
                        bass.AP(
                            src_sb[:].tensor,
                            src_sb[:].offset + c0 * src_sb[:].ap[0][0]
                            + cc * SLABP,
                            [[src_sb[:].ap[0][0], nch], [1, NROW * 31]],
                        ),
                    )
                    if side == 1:
                        xload_half(cc, h)

            def xload_half(cc, h):
                """DRAM staging -> one X half tile (rows + bias row)."""
                xc0 = cc * XBLK
                w = CCW[cc] * 31
                h0 = h * 1984
                hw = min(1984, w - h0)
                xt = xts[cc][h]
                nc.sync.dma_start(
                    xt[0 : XROWS - 1, 0:hw],
                    x_dram[:, xc0 + h0 : xc0 + h0 + hw],
                )
                nc.sync.dma_start(
                    xt[XROWS - 1 : XROWS, 0:hw],
                    bias_d[0:1, xc0 + h0 : xc0 + h0 + hw],
                )

            def xalloc(cc):
                w = CCW[cc] * 31
                return [xpool.tile([XROWS, 1984], bf16, tag="x",
                                   name=f"xt{cc}_{h0}")
                        for h0 in range(0, w, 1984)]

            def pw_matmuls(cc, xt, mi, g):
                subs, _ = _sub_plan(cc)
                moff, msz = MCH[mi]
                pw = pwps.tile([125, 2, 512], f32, tag="pw")
                for j, si in enumerate(g):
                    soff, sw = subs[si]
                    xh = xt[soff // 1984]
                    nc.tensor.matmul(
                        pw[0:msz, j : j + 1, 0:sw],
                        lhsT=a_sb[:, moff : moff + msz],
                        rhs=xh[:, soff % 1984 : soff % 1984 + sw],
                        start=True,
                        stop=True,
                    )
                ng = len(g)
                gsw = subs[g[0]][1]
                return pw[0:msz, 0:ng, 0:gsw].rearrange(
                    "p g (c l) -> p g c l", l=31
                )

            def grp_info(cc, mi, gi):
                subs, groups = _sub_plan(cc)
                g = groups[gi]
                gsw = subs[g[0]][1]
                gc = gsw // 31
                cb = CC0[cc] + subs[g[0]][0] // 31
                return g, gc, len(g) * gc, cb

            def pairwise_direct(cc, xt, mi, gi, reds):
                g, gc, ncols, cb = grp_info(cc, mi, gi)
                moff, msz = MCH[mi]
                pwr = pw_matmuls(cc, xt, mi, g)
                nc.vector.tensor_reduce(
                    reds[mi][0:msz, cb : cb + ncols],
                    pwr,
                    axis=mybir.AxisListType.X,
                    op=mybir.AluOpType.max,
                )

            def pairwise_b(cc, xt, mi, gis, reds):
                """B route for up to 4 adjacent groups: Act drains each psum
                tile to SBUF bf16 (relu folded in -- idempotent under the
                later finish relu), then DVE max-trees 31->16->8->4 in 2x
                mode and a small reduce writes the result tile."""
                moff, msz = MCH[mi]
                infos = [grp_info(cc, mi, gi) for gi in gis]
                tg = sum(len(i[0]) for i in infos)
                gc = infos[0][1]
                cb = infos[0][3]
                ncols = sum(i[2] for i in infos)
                sc2 = scr2pool.tile([125, 8, 16, 48], mybir.dt.bfloat16,
                                    tag="scr2")
                goff = 0
                for g, _, _, _ in infos:
                    pwr = pw_matmuls(cc, xt, mi, g)
                    nc.scalar.activation(
                        sc2[0:msz, goff : goff + len(g), 0:gc, 0:31],
                        pwr,
                        mybir.ActivationFunctionType.Relu,
                    )
                    goff += len(g)

                def _stage2():
                    s = sc2[0:msz, 0:tg, 0:gc, :]
                    nc.vector.tensor_tensor(
                        s[:, :, :, 32:48], s[:, :, :, 0:16], s[:, :, :, 15:31],
                        op=mybir.AluOpType.max,
                    )
                    nc.vector.tensor_tensor(
                        s[:, :, :, 0:8], s[:, :, :, 32:40], s[:, :, :, 40:48],
                        op=mybir.AluOpType.max,
                    )
                    nc.vector.tensor_tensor(
                        s[:, :, :, 8:12], s[:, :, :, 0:4], s[:, :, :, 4:8],
                        op=mybir.AluOpType.max,
                    )
                    nc.vector.tensor_reduce(
                        reds[mi][0:msz, cb : cb + ncols],
                        s[:, :, :, 8:12],
                        axis=mybir.AxisListType.X,
                        op=mybir.AluOpType.max,
                    )
                    return None

                return _stage2

            reds = [
                redpool.tile([125, NCH], f32, tag=f"red{i}", name=f"red{i}")
                for i in range(5)
            ]

            def pw_units(cc):
                units = []
                for mi in range(5):
                    d, b = routes_for(cc, mi)
                    for gi in d:
                        units.append(("d", cc, mi, (gi,)))
                    for q in b:
                        units.append(("b", cc, mi, q))
                return units

            def finish_cc_mi(cc, mi):
                # relu (only over DVE-direct columns -- the Act-routed ones
                # were relu'd in the psum drain copy) + output DMA for one
                # pair-tile, one channel chunk's columns
                moff, msz = MCH[mi]
                c0 = CC0[cc]
                c1 = min(c0 + CCW[cc], NCH)
                d, _ = routes_for(cc, mi)
                if d:
                    r0 = CC0[cc] + grp_info(cc, mi, min(d))[3] - CC0[cc]
                    rw = sum(grp_info(cc, mi, gi)[2] for gi in d)
                    rbase = grp_info(cc, mi, min(d))[3]
                    nc.scalar.activation(
                        reds[mi][0:msz, rbase : rbase + rw],
                        reds[mi][0:msz, rbase : rbase + rw],
                        mybir.ActivationFunctionType.Relu,
                    )
                nc.sync.dma_start(
                    out_d[moff : moff + msz, c0:c1], reds[mi][0:msz, c0:c1]
                )

            xts = {}
            tails = []      # stage-2 closures (GPSIMD work)
            tails3 = []     # stage-3 closures (DVE finals)

            def emit(kind, cc, mi, gis):
                if kind == "d":
                    pairwise_direct(cc, xts[cc], mi, gis[0], reds)
                    return
                tails.append(pairwise_b(cc, xts[cc], mi, gis, reds))

            def flush_tails():
                for t in tails3:
                    t()
                tails3.clear()
                for t in tails:
                    n = t()
                    if n is not None:
                        tails3.append(n)
                tails.clear()

            # conv cc1 first; prologue DMAs land under the warmup
            xts[1] = xalloc(1)
            for side in (0, 1):
                for pci in (0, 1):
                    conv_unit(1, side, pci)
            # conv cc0 interleaved with pairwise cc1
            xts[0] = xalloc(0)
            q1 = pw_units(1)
            sched0 = [3, 3, 3, 2]
            qi = 0
            for ui, (side, pci) in enumerate(
                ((0, 0), (0, 1), (1, 0), (1, 1))
            ):
                conv_unit(0, side, pci)
                flush_tails()
                for u in q1[qi : qi + sched0[ui]]:
                    emit(*u)
                qi += sched0[ui]
            for u in q1[qi:]:
                emit(*u)
            flush_tails()
            flush_tails()
            for mi in range(5):
                finish_cc_mi(1, mi)
            # conv cc2 interleaved with pairwise cc0; hold some cc0 groups
            # back as PE filler over the xload(2) latency
            xts[2] = xalloc(2)
            q0 = pw_units(0)
            sched2 = [1, 1, 1, 1]
            qi = 0
            for ui, (side, pci) in enumerate(
                ((0, 0), (0, 1), (1, 0), (1, 1))
            ):
                conv_unit(2, side, pci)
                flush_tails()
                for u in q0[qi : qi + sched2[ui]]:
                    emit(*u)
                qi += sched2[ui]
            for u in q0[qi:]:
                emit(*u)
            flush_tails()
            flush_tails()
            for mi in range(5):
                finish_cc_mi(0, mi)
            # tail: pairwise cc2 (direct DVE reduces, shortest chains)
            for mi in range(5):
                d2, b2 = routes_for(2, mi)
                for q in b2:
                    emit("b", 2, mi, q)
                for gi in d2:
                    emit("d", 2, mi, (gi,))
                flush_tails()
                if mi > 1:
                    finish_cc_mi(2, mi - 2)
            flush_tails()
            flush_tails()
            finish_cc_mi(2, 3)
            finish_cc_mi(2, 4)

    nc.compile()
    return nc


def get_program():
    global _PROG
    if _PROG is None:
        _PROG = _build_program()
    return _PROG


def build_inputs(s, q, ws, bs):
    """Host-side shard prep. ws/bs: dicts k -> w(150, 1024, k) / b(150,).

    Returns in_maps. Core c handles episode c//2, channel half c%2
    (half 0 = out-channels 0:75 of each conv, half 1 = 75:150).
    """
    s = np.asarray(s, dtype=np.float32).reshape(B, NROW, L, D)
    q = np.asarray(q, dtype=np.float32).reshape(B, NQROW, L, D)

    walls, biasxs = [], []
    for half in range(2):
        wall = np.zeros((D, WTOT), dtype=np.float32)
        bias_dev = np.zeros(NCH, dtype=np.float32)
        for k in (2, 3, 4, 5):
            g = ORD_OF_K[k]
            bias_dev[g * 75 : (g + 1) * 75] = bs[k][75 * half : 75 * half + 75]
        for di, (delta, sz) in enumerate(DELTAS):
            for side in range(2):
                wd = np.zeros((D, NCH), dtype=np.float32)
                for k in (2, 3, 4, 5):
                    t = delta + PAD_OF_K[k]
                    if not (0 <= t < k):
                        continue
                    g = ORD_OF_K[k]
                    wd[:, g * 75 : (g + 1) * 75] = ws[k][
                        75 * half : 75 * half + 75, side * D : (side + 1) * D, t
                    ].T
                for cc in range(3):
                    c0 = CC0[cc]
                    if sz <= c0:
                        continue
                    w = min(CCW[cc], sz - c0)
                    col = WCOL[(cc, side, di)]
                    wall[:, col : col + w] = wd[:, c0 : c0 + w]
        biasx = np.zeros(3 * XBLK, dtype=np.float32)
        for ch in range(NCH):
            cc = 2 if ch >= 256 else ch // 128
            p = cc * XBLK + (ch - CC0[cc]) * 31
            biasx[p : p + 31] = bias_dev[ch]
        walls.append(wall.astype(BF16))
        biasxs.append(biasx[None, :].astype(BF16))

    amat = np.zeros((XROWS, NPAIR), dtype=np.float32)
    for i in range(NROW):
        for j in range(NQROW):
            p = i * NQROW + j
            amat[i, p] = 1.0
            amat[NROW + j, p] = 1.0
    amat[XROWS - 1, :] = 1.0
    amat = amat.astype(BF16)

    psas, pqas = [], []
    for b in range(B):
        psa = np.zeros((D, P_COLS), dtype=np.float32)
        pqa = np.zeros((D, P_COLS), dtype=np.float32)
        for r in range(NROW):
            psa[:, r * ROWSTR + 4 : r * ROWSTR + 4 + L] = s[b, r].T
        for r in range(NQROW):
            pqa[:, r * ROWSTR + 4 : r * ROWSTR + 4 + L] = q[b, r].T
        psas.append(psa.astype(BF16))
        pqas.append(pqa.astype(BF16))

    in_maps = []
    for core in range(8):
        b, half = core // 2, core % 2
        in_maps.append(
            {
                "ps": psas[b],
                "pq": pqas[b],
                "w": walls[half],
                "a": amat,
                "bias": biasxs[half],
            }
        )
    return in_maps


# device channel -> original output-column map (an involution)
_IDX = np.array(
    [(3 - g) * 75 + u for g in range(4) for u in range(75)], dtype=np.int64
)


def assemble_outputs(core_outs):
    """core_outs: list of 8 arrays [NPAIR, NCH] -> (s_out, q_out)."""
    s_out = np.empty((B, NPAIR, 300), dtype=np.float32)
    q_out = np.empty((B, NPAIR, 300), dtype=np.float32)
    for core in range(8):
        b, half = core // 2, core % 2
        arr = np.ascontiguousarray(core_outs[core])
        dst = s_out if half == 0 else q_out
        dst[b] = arr[:, _IDX]
    return s_out.reshape(-1, 300), q_out.reshape(-1, 300)


def kernel(s, q, w2, b2, w3, b3, w4, b4, w5, b5, B=4, N=5, K=5, Q=5, L=31):
    ws = {2: np.asarray(w2, np.float32), 3: np.asarray(w3, np.float32),
          4: np.asarray(w4, np.float32), 5: np.asarray(w5, np.float32)}
    bs = {2: np.asarray(b2, np.float32), 3: np.asarray(b3, np.float32),
          4: np.asarray(b4, np.float32), 5: np.asarray(b5, np.float32)}
    in_maps = build_inputs(s, q, ws, bs)
    nc = get_program()
    res = run_bass_kernel_spmd(nc, in_maps, list(range(8))).results
    return assemble_outputs([res[c]["out"] for c in range(8)])


# revision 75
# speedup vs baseline: 1.4368x; 1.0013x over previous
"""Trainium2 Bass kernel for nn_CNNInteractLayer (CNN interaction layer).

Math: for each episode b, s-row i, q-row j:
  out[b,i,j] = maxpool_L(relu(conv_k(concat(s[b,i], q[b,j])))) for k in 2..5
Key factorization: conv(concat(s,q)) = conv_s(s) + conv_q(q) + bias, so we
compute per-row convolutions once (25 s + 25 q rows per core instead of 625
pairs) and form pairwise sums with a 0/1 selection matmul on the PE. The max
over the L=31 window is split between DVE (direct tensor_reduce from PSUM)
and Act+DVE (relu-folded psum->SBUF bf16 copy, then a 2x-mode max tree),
keeping both PSUM-capable engines saturated.

Sharding: 8 cores = 4 episodes x 2 halves of the 600 conv output channels
(half 0 -> the s_out channels of each conv, half 1 -> the q_out channels).
Each core computes all 625 row pairs of its episode for its 300 channels;
no inter-core communication. All matmul operands are bf16 (PSUM stays fp32).
"""

import os
import sys

import numpy as np

for _p in ("/opt/trn_rl_repo",):
    if os.path.isdir(_p) and _p not in sys.path:
        sys.path.insert(0, _p)

# the bass runner needs the axon jax backend; don't let a cpu-only pin hide it
if "axon" not in os.environ.get("JAX_PLATFORMS", "axon"):
    os.environ.pop("JAX_PLATFORMS", None)

import ml_dtypes  # noqa: E402

from concourse import bacc, bass, mybir, tile  # noqa: E402
from concourse.bass_utils import run_bass_kernel_spmd  # noqa: E402

BF16 = ml_dtypes.bfloat16

# Problem dims (hardcoded per spec)
B, N, K, Q, L, D = 4, 5, 5, 5, 31, 512
NROW = 25               # s-rows per episode
NQROW = 25              # q-rows per episode
ROWSTR = L + 4          # padded row stride (pad 2 each side)
SLAB = 876              # halo'd conv output span per position chunk
SLABP = 775             # packed slab width per channel chunk (25*31)
P_COLS = 880            # input halo: 2 left + enough right for pos 875
NCH = 300               # device channels per core: [k5 | k4 | k3 | k2] x 75
NPAIR = NROW * NQROW    # 625
XROWS = NROW + NQROW + 1  # 25 s + 25 q + 1 bias
XBLK = 128 * 31         # x staging stride per channel chunk
# delta (tap shift) groups; prefix-size in device channel order
DELTAS = [(-2, 150), (-1, 300), (0, 300), (1, 225), (2, 75)]
CC0 = [0, 128, 256]     # channel chunk starts
CCW = [128, 128, 44]
POSCH = [(0, 488), (488, 388)]  # conv position chunks (PSUM bank = 488 f32)
SUBW = 496              # pairwise n-subchunk: 16 channel groups
MCH = [(125 * i, 125) for i in range(5)]  # pair-tile chunks
PAD_OF_K = {2: 1, 3: 1, 4: 2, 5: 2}
ORD_OF_K = {5: 0, 4: 1, 3: 2, 2: 3}

# per-group reduce routes (GPSIMD is unusable on TRN2 for this: no PSUM
# access, no TensorTensor, InstPool is DVE-only):
#  direct: DVE reduces the psum group in one go
#  B: Act copies psum->SBUF bf16 (relu folded in), DVE runs a 2x-mode max
#     tree (31->16->8->4) + tiny reduce; a quad (4 adjacent groups = one
#     128-ch chunk) shares one tree
def routes_for(cc, mi):
    """(directs, b_units) for one pair-tile of one channel chunk."""
    if cc == 0:
        return [0], [(1, 2, 3)]
    if cc == 1:
        return ([], [(0, 1, 2, 3)]) if mi <= 1 else ([0, 1], [(2, 3)])
    return [1], [(0,)]


# chunk-major packed-W layout: per channel chunk, [side s | side q], each a
# concatenation of the valid delta groups' column slices for that chunk
def _chunk_tables():
    chw = []          # per-side width of each chunk block
    coloff = {}       # (cc, side, di) -> column offset in packed W
    off = 0
    for cc in range(3):
        c0 = CC0[cc]
        widths = []
        for _, sz in DELTAS:
            widths.append(min(CCW[cc], sz - c0) if sz > c0 else 0)
        side_w = sum(widths)
        for side in range(2):
            p = off + side * side_w
            for di, w in enumerate(widths):
                if w:
                    coloff[(cc, side, di)] = p
                    p += w
        chw.append(side_w)
        off += 2 * side_w
    return chw, coloff


CHW, WCOL = _chunk_tables()
CHOFF = [sum(2 * w for w in CHW[:i]) for i in range(4)]
WSIDE = sum(CHW)        # 1050
WTOT = 2 * WSIDE        # 2100

_PROG = None


def _sub_plan(cc):
    """(offset, width) n-subchunks within an X chunk + psum tile grouping."""
    total = CCW[cc] * 31
    subs = []
    off = 0
    while off < total:
        w = min(SUBW, total - off)
        subs.append((off, w))
        off += w
    # groups of <=2 subchunks sharing one psum tile; equal width within group
    groups = []
    i = 0
    while i < len(subs):
        g = [i]
        while (
            len(g) < 2
            and i + len(g) < len(subs)
            and subs[i + len(g)][1] == subs[i][1]
        ):
            g.append(i + len(g))
        groups.append(g)
        i += len(g)
    return subs, groups


def _build_program():
    nc = bacc.Bacc("TRN2", target_bir_lowering=False, debug=False, num_devices=8)
    f32 = mybir.dt.float32
    f32r = mybir.dt.float32r
    bf16 = mybir.dt.bfloat16

    ps_d = nc.dram_tensor("ps", [D, P_COLS], bf16, kind="ExternalInput")
    pq_d = nc.dram_tensor("pq", [D, P_COLS], bf16, kind="ExternalInput")
    w_d = nc.dram_tensor("w", [D, WTOT], bf16, kind="ExternalInput")
    a_d = nc.dram_tensor("a", [XROWS, NPAIR], bf16, kind="ExternalInput")
    bias_d = nc.dram_tensor("bias", [1, 3 * XBLK], bf16, kind="ExternalInput")
    x_dram = nc.dram_tensor("xstage", [XROWS - 1, 3 * XBLK], bf16)
    out_d = nc.dram_tensor("out", [NPAIR, NCH], f32, kind="ExternalOutput")

    with tile.TileContext(nc) as tc:
        with (
            tc.tile_pool(name="persist", bufs=1) as big,
            tc.tile_pool(name="xpool", bufs=4) as xpool,
            tc.tile_pool(name="redpool", bufs=5) as redpool,
            tc.tile_pool(name="scrpool", bufs=4) as scrpool,
            tc.tile_pool(name="scr2pool", bufs=4) as scr2pool,
            tc.tile_pool(name="convps", bufs=2, space="PSUM") as convps,
            tc.tile_pool(name="pwps", bufs=3, space="PSUM") as pwps,
        ):
            w_sb = big.tile([128, 4 * WTOT], bf16, tag="w")
            ps_sb = big.tile([128, 4 * P_COLS], bf16, tag="ps")
            pq_sb = big.tile([128, 4 * P_COLS], bf16, tag="pq")
            cs_sb = big.tile([128, 3 * SLABP], bf16, tag="cs")
            cq_sb = big.tile([128, 3 * SLABP], bf16, tag="cq")
            a_sb = big.tile([XROWS, NPAIR], bf16, tag="a")

            # keep the PE busy during the input-DMA prologue so the HAM
            # clock gate is warm (2.4 GHz) when the first conv matmul lands
            warm_sb = big.tile([128, 512], bf16, tag="warm")
            warm_ps = convps.tile([128, 492], f32, tag="conv")
            nc.vector.memset(warm_sb[:, 0:256], 0.0)
            for _wi in range(18):
                nc.tensor.matmul(
                    warm_ps[0:128, 0:256],
                    lhsT=warm_sb[:, 0:128],
                    rhs=warm_sb[:, 0:256],
                    start=True,
                    stop=True,
                )

            def wload(cc):
                wd = w_d[:].rearrange("(d p) c -> p d c", p=128)
                ws = w_sb[:].rearrange("p (d c) -> p d c", c=WTOT)
                nc.sync.dma_start(
                    ws[:, :, CHOFF[cc] : CHOFF[cc + 1]],
                    wd[:, :, CHOFF[cc] : CHOFF[cc + 1]],
                )

            ps3 = ps_sb[:].rearrange("p (d c) -> p d c", c=P_COLS)
            pd3 = ps_d[:].rearrange("(d p) c -> p d c", p=128)
            nc.sync.dma_start(ps3[:, :, 0:496], pd3[:, :, 0:496])
            nc.sync.dma_start(ps3[:, :, 496:P_COLS], pd3[:, :, 496:P_COLS])
            # cc1's weights split per d-chunk so the first conv matmul (d0)
            # only waits on a quarter of the load
            wd_r = w_d[:].rearrange("(d p) c -> p d c", p=128)
            ws_r = w_sb[:].rearrange("p (d c) -> p d c", c=WTOT)
            for _d in range(4):
                nc.sync.dma_start(
                    ws_r[:, _d : _d + 1, CHOFF[1] : CHOFF[2]],
                    wd_r[:, _d : _d + 1, CHOFF[1] : CHOFF[2]],
                )
            nc.sync.dma_start(
                pq_sb[:].rearrange("p (d c) -> p d c", c=P_COLS),
                pq_d[:].rearrange("(d p) c -> p d c", p=128),
            )
            nc.sync.dma_start(a_sb[:], a_d[:])
            wload(0)
            wload(2)

            def conv_unit(cc, side, pci):
                """Conv for channel chunk cc, one side, one position chunk.

                """
                pos0, pwid = POSCH[pci]
                src = ps_sb if side == 0 else pq_sb
                c0, ccw = CC0[cc], CCW[cc]
                mms = []
                for d in range(4):
                    for di, (delta, sz) in enumerate(DELTAS):
                        if sz <= c0:
                            continue
                        mms.append((d, di, delta, min(ccw, sz - c0)))
                # first and last matmul of the accumulation group must cover
                # the full partition range (start/stop are per-element)
                full = [m for m in mms if m[3] == ccw]
                part = [m for m in mms if m[3] < ccw]
                mms = [full[0]] + part + full[1:]
                psum = convps.tile([128, 492], f32, tag="conv")
                for idx, (d, di, delta, wcc) in enumerate(mms):
                    lcol = d * WTOT + WCOL[(cc, side, di)]
                    rcol = d * P_COLS + pos0 + delta + 2
                    nc.tensor.matmul(
                        psum[0:wcc, 0:pwid],
                        lhsT=w_sb[:, lcol : lcol + wcc],
                        rhs=src[:, rcol : rcol + pwid],
                        start=(idx == 0),
                        stop=(idx == len(mms) - 1),
                    )
                dst = cs_sb if side == 0 else cq_sb
                # packed copy: strip the 4-col inter-row halo while draining
                r0, nr, lc = (0, 14, 2) if pci == 0 else (14, 11, 4)
                pr = psum[0:ccw, lc : lc + nr * ROWSTR].rearrange(
                    "p (r s) -> p r s", s=ROWSTR
                )
                nc.scalar.copy(
                    dst[0:ccw, cc * SLABP + r0 * 31 :
                        cc * SLABP + (r0 + nr) * 31],
                    pr[:, :, 0:31],
                )
                if pci == 1:
                    xevict_side(cc, side)

            def xevict_side(cc, side):
                """one side's conv slab -> DRAM staging X[row, slot*31+l],
                split per channel half so each X half-tile's load chain
                starts as soon as its own half is staged."""
                xc0 = cc * XBLK
                ccw = CCW[cc]
                src_sb = cs_sb if side == 0 else cq_sb
                for h in range((ccw + 63) // 64):
                    c0 = h * 64
                    nch = min(64, ccw - c0)
                    nc.sync.dma_start(
                        bass.AP(
                            x_dram[:].tensor,
                            side * NROW * 3 * XBLK + xc0 + c0 * 31,
                            [[31, nch], [3 * XBLK, NROW], [1, 31]],
                        ),
                        bass.AP(
                            src_sb[:].tensor,
                            src_sb[:].offset + c0 * src_sb[:].ap[0][0]
                            + cc * SLABP,
                            [[src_sb[:].ap[0][0], nch], [1, NROW * 31]],
                        ),
                    )
                    if side == 1:
                        xload_half(cc, h)

            def xload_half(cc, h):
                """DRAM staging -> one X half tile (rows + bias row)."""
                xc0 = cc * XBLK
                w = CCW[cc] * 31
                h0 = h * 1984
                hw = min(1984, w - h0)
                xt = xts[cc][h]
                nc.sync.dma_start(
                    xt[0 : XROWS - 1, 0:hw],
                    x_dram[:, xc0 + h0 : xc0 + h0 + hw],
                )
                nc.sync.dma_start(
                    xt[XROWS - 1 : XROWS, 0:hw],
                    bias_d[0:1, xc0 + h0 : xc0 + h0 + hw],
                )

            def xalloc(cc):
                w = CCW[cc] * 31
                return [xpool.tile([XROWS, 1984], bf16, tag="x",
                                   name=f"xt{cc}_{h0}")
                        for h0 in range(0, w, 1984)]

            def pw_matmuls(cc, xt, mi, g):
                subs, _ = _sub_plan(cc)
                moff, msz = MCH[mi]
                pw = pwps.tile([125, 2, 512], f32, tag="pw")
                for j, si in enumerate(g):
                    soff, sw = subs[si]
                    xh = xt[soff // 1984]
                    nc.tensor.matmul(
                        pw[0:msz, j : j + 1, 0:sw],
                        lhsT=a_sb[:, moff : moff + msz],
                        rhs=xh[:, soff % 1984 : soff % 1984 + sw],
                        start=True,
                        stop=True,
                    )
                ng = len(g)
                gsw = subs[g[0]][1]
                return pw[0:msz, 0:ng, 0:gsw].rearrange(
                    "p g (c l) -> p g c l", l=31
                )

            def grp_info(cc, mi, gi):
                subs, groups = _sub_plan(cc)
                g = groups[gi]
                gsw = subs[g[0]][1]
                gc = gsw // 31
                cb = CC0[cc] + subs[g[0]][0] // 31
                return g, gc, len(g) * gc, cb

            def pairwise_direct(cc, xt, mi, gi, reds):
                g, gc, ncols, cb = grp_info(cc, mi, gi)
                moff, msz = MCH[mi]
                pwr = pw_matmuls(cc, xt, mi, g)
                nc.vector.tensor_reduce(
                    reds[mi][0:msz, cb : cb + ncols],
                    pwr,
                    axis=mybir.AxisListType.X,
                    op=mybir.AluOpType.max,
                )

            def pairwise_b(cc, xt, mi, gis, reds):
                """B route for up to 4 adjacent groups: Act drains each psum
                tile to SBUF bf16 (relu folded in -- idempotent under the
                later finish relu), then DVE max-trees 31->16->8->4 in 2x
                mode and a small reduce writes the result tile."""
                moff, msz = MCH[mi]
                infos = [grp_info(cc, mi, gi) for gi in gis]
                tg = sum(len(i[0]) for i in infos)
                gc = infos[0][1]
                cb = infos[0][3]
                ncols = sum(i[2] for i in infos)
                sc2 = scr2pool.tile([125, 8, 16, 48], mybir.dt.bfloat16,
                                    tag="scr2")
                goff = 0
                for g, _, _, _ in infos:
                    pwr = pw_matmuls(cc, xt, mi, g)
                    nc.scalar.activation(
                        sc2[0:msz, goff : goff + len(g), 0:gc, 0:31],
                        pwr,
                        mybir.ActivationFunctionType.Relu,
                    )
                    goff += len(g)

                def _stage2():
                    s = sc2[0:msz, 0:tg, 0:gc, :]
                    nc.vector.tensor_tensor(
                        s[:, :, :, 32:48], s[:, :, :, 0:16], s[:, :, :, 15:31],
                        op=mybir.AluOpType.max,
                    )
                    nc.vector.tensor_tensor(
                        s[:, :, :, 0:8], s[:, :, :, 32:40], s[:, :, :, 40:48],
                        op=mybir.AluOpType.max,
                    )
                    nc.vector.tensor_tensor(
                        s[:, :, :, 8:12], s[:, :, :, 0:4], s[:, :, :, 4:8],
                        op=mybir.AluOpType.max,
                    )
                    nc.vector.tensor_reduce(
                        reds[mi][0:msz, cb : cb + ncols],
                        s[:, :, :, 8:12],
                        axis=mybir.AxisListType.X,
                        op=mybir.AluOpType.max,
                    )
                    return None

                return _stage2

            reds = [
                redpool.tile([125, NCH], f32, tag=f"red{i}", name=f"red{i}")
                for i in range(5)
            ]

            def pw_units(cc):
                units = []
                for mi in range(5):
                    d, b = routes_for(cc, mi)
                    for gi in d:
                        units.append(("d", cc, mi, (gi,)))
                    for q in b:
                        units.append(("b", cc, mi, q))
                return units

            def finish_cc_mi(cc, mi):
                # relu (only over DVE-direct columns -- the Act-routed ones
                # were relu'd in the psum drain copy) + output DMA for one
                # pair-tile, one channel chunk's columns
                moff, msz = MCH[mi]
                c0 = CC0[cc]
                c1 = min(c0 + CCW[cc], NCH)
                d, _ = routes_for(cc, mi)
                if d:
                    r0 = CC0[cc] + grp_info(cc, mi, min(d))[3] - CC0[cc]
                    rw = sum(grp_info(cc, mi, gi)[2] for gi in d)
                    rbase = grp_info(cc, mi, min(d))[3]
                    nc.scalar.activation(
                        reds[mi][0:msz, rbase : rbase + rw],
                        reds[mi][0:msz, rbase : rbase + rw],
                        mybir.ActivationFunctionType.Relu,
                    )
                nc.sync.dma_start(
                    out_d[moff : moff + msz, c0:c1], reds[mi][0:msz, c0:c1]
                )

            xts = {}
            tails = []      # stage-2 closures (GPSIMD work)
            tails3 = []     # stage-3 closures (DVE finals)

            def emit(kind, cc, mi, gis):
                if kind == "d":
                    pairwise_direct(cc, xts[cc], mi, gis[0], reds)
                    return
                tails.append(pairwise_b(cc, xts[cc], mi, gis, reds))

            def flush_tails():
                for t in tails3:
                    t()
                tails3.clear()
                for t in tails:
                    n = t()
                    if n is not None:
                        tails3.append(n)
                tails.clear()

            # conv cc1 first; prologue DMAs land under the warmup
            xts[1] = xalloc(1)
            for side in (0, 1):
                for pci in (0, 1):
                    conv_unit(1, side, pci)
            # conv cc0 interleaved with pairwise cc1
            xts[0] = xalloc(0)
            q1 = pw_units(1)
            sched0 = [3, 3, 3, 2]
            qi = 0
            for ui, (side, pci) in enumerate(
                ((0, 0), (0, 1), (1, 0), (1, 1))
            ):
                conv_unit(0, side, pci)
                flush_tails()
                for u in q1[qi : qi + sched0[ui]]:
                    emit(*u)
                qi += sched0[ui]
            for u in q1[qi:]:
                emit(*u)
            flush_tails()
            flush_tails()
            for mi in range(5):
                finish_cc_mi(1, mi)
            # conv cc2 interleaved with pairwise cc0; hold some cc0 groups
            # back as PE filler over the xload(2) latency
            xts[2] = xalloc(2)
            q0 = pw_units(0)
            sched2 = [1, 1, 1, 1]
            qi = 0
            for ui, (side, pci) in enumerate(
                ((0, 0), (0, 1), (1, 0), (1, 1))
            ):
                conv_unit(2, side, pci)
                flush_tails()
                for u in q0[qi : qi + sched2[ui]]:
                    emit(*u)
                qi += sched2[ui]
            for u in q0[qi:]:
                emit(*u)
            flush_tails()
            flush_tails()
            for mi in range(5):
                finish_cc_mi(0, mi)
            # tail: pairwise cc2 (direct DVE reduces, shortest chains)
            for mi in range(5):
                d2, b2 = routes_for(2, mi)
                for q in b2:
                    emit("b", 2, mi, q)
                for gi in d2:
                    emit("d", 2, mi, (gi,))
                flush_tails()
                if mi > 0:
                    finish_cc_mi(2, mi - 1)
            flush_tails()
            flush_tails()
            finish_cc_mi(2, 4)

    nc.compile()
    return nc


def get_program():
    global _PROG
    if _PROG is None:
        _PROG = _build_program()
    return _PROG


def build_inputs(s, q, ws, bs):
    """Host-side shard prep. ws/bs: dicts k -> w(150, 1024, k) / b(150,).

    Returns in_maps. Core c handles episode c//2, channel half c%2
    (half 0 = out-channels 0:75 of each conv, half 1 = 75:150).
    """
    s = np.asarray(s, dtype=np.float32).reshape(B, NROW, L, D)
    q = np.asarray(q, dtype=np.float32).reshape(B, NQROW, L, D)

    walls, biasxs = [], []
    for half in range(2):
        wall = np.zeros((D, WTOT), dtype=np.float32)
        bias_dev = np.zeros(NCH, dtype=np.float32)
        for k in (2, 3, 4, 5):
            g = ORD_OF_K[k]
            bias_dev[g * 75 : (g + 1) * 75] = bs[k][75 * half : 75 * half + 75]
        for di, (delta, sz) in enumerate(DELTAS):
            for side in range(2):
                wd = np.zeros((D, NCH), dtype=np.float32)
                for k in (2, 3, 4, 5):
                    t = delta + PAD_OF_K[k]
                    if not (0 <= t < k):
                        continue
                    g = ORD_OF_K[k]
                    wd[:, g * 75 : (g + 1) * 75] = ws[k][
                        75 * half : 75 * half + 75, side * D : (side + 1) * D, t
                    ].T
                for cc in range(3):
                    c0 = CC0[cc]
                    if sz <= c0:
                        continue
                    w = min(CCW[cc], sz - c0)
                    col = WCOL[(cc, side, di)]
                    wall[:, col : col + w] = wd[:, c0 : c0 + w]
        biasx = np.zeros(3 * XBLK, dtype=np.float32)
        for ch in range(NCH):
            cc = 2 if ch >= 256 else ch // 128
            p = cc * XBLK + (ch - CC0[cc]) * 31
            biasx[p : p + 31] = bias_dev[ch]
        walls.append(wall.astype(BF16))
        biasxs.append(biasx[None, :].astype(BF16))

    amat = np.zeros((XROWS, NPAIR), dtype=np.float32)
    for i in range(NROW):
        for j in range(NQROW):
            p = i * NQROW + j
            amat[i, p] = 1.0
            amat[NROW + j, p] = 1.0
    amat[XROWS - 1, :] = 1.0
    amat = amat.astype(BF16)

    psas, pqas = [], []
    for b in range(B):
        psa = np.zeros((D, P_COLS), dtype=np.float32)
        pqa = np.zeros((D, P_COLS), dtype=np.float32)
        for r in range(NROW):
            psa[:, r * ROWSTR + 4 : r * ROWSTR + 4 + L] = s[b, r].T
        for r in range(NQROW):
            pqa[:, r * ROWSTR + 4 : r * ROWSTR + 4 + L] = q[b, r].T
        psas.append(psa.astype(BF16))
        pqas.append(pqa.astype(BF16))

    in_maps = []
    for core in range(8):
        b, half = core // 2, core % 2
        in_maps.append(
            {
                "ps": psas[b],
                "pq": pqas[b],
                "w": walls[half],
                "a": amat,
                "bias": biasxs[half],
            }
        )
    return in_maps


# device channel -> original output-column map (an involution)
_IDX = np.array(
    [(3 - g) * 75 + u for g in range(4) for u in range(75)], dtype=np.int64
)


def assemble_outputs(core_outs):
    """core_outs: list of 8 arrays [NPAIR, NCH] -> (s_out, q_out)."""
    s_out = np.empty((B, NPAIR, 300), dtype=np.float32)
    q_out = np.empty((B, NPAIR, 300), dtype=np.float32)
    for core in range(8):
        b, half = core // 2, core % 2
        arr = np.ascontiguousarray(core_outs[core])
        dst = s_out if half == 0 else q_out
        dst[b] = arr[:, _IDX]
    return s_out.reshape(-1, 300), q_out.reshape(-1, 300)


def kernel(s, q, w2, b2, w3, b3, w4, b4, w5, b5, B=4, N=5, K=5, Q=5, L=31):
    ws = {2: np.asarray(w2, np.float32), 3: np.asarray(w3, np.float32),
          4: np.asarray(w4, np.float32), 5: np.asarray(w5, np.float32)}
    bs = {2: np.asarray(b2, np.float32), 3: np.asarray(b3, np.float32),
          4: np.asarray(b4, np.float32), 5: np.asarray(b5, np.float32)}
    in_maps = build_inputs(s, q, ws, bs)
    nc = get_program()
    res = run_bass_kernel_spmd(nc, in_maps, list(range(8))).results
    return assemble_outputs([res[c]["out"] for c in range(8)])
